# revision 1
# baseline (speedup 1.0000x reference)
"""Trainium2 Bass kernel for a CQT (constant-Q transform) nn.Module.

Reference computation (per batch sample b, channel c):
    out[b, c, k, f, 0] = sum_t x[b, c, f*HOP + t] * w_re[k, t]
    out[b, c, k, f, 1] = sum_t x[b, c, f*HOP + t] * w_im[k, t]
where w_re/w_im are Hann-windowed complex exponentials with per-bin ragged
lengths (longest 11340 samples), HOP=512, 84 bins, 409 frames.

Strategy: data-parallel over the batch (1 sample per NeuronCore, 8 cores).
Per core the correlation is a banded matmul: the contraction axis t is split
into 89 chunks of 128; chunk c needs x samples x[(f + c//4)*512 + (c%4)*128 + r].
The signal is laid out once in SBUF as Xt[r, ch, rc, m] = x[ch, m*512+rc*128+r]
so every chunk's moving operand is a 410-column slice of a resident tile
(410 = 409 frames padded to the even count fp32r requires).

Weight rows are interleaved (re_k, im_k) pairs sorted by descending window
length, so the rows active in a chunk are always a prefix. Rows 0..127
(bins 0..63) form accumulation group G1 (89 chunks); rows 128..167
(bins 64..83, windows <= 281 samples) form group G2 (3 chunks). Weights are
stored column-compacted (only active rows per chunk): 1.2 MB of DMA instead
of 7.9 MB, with no change in matmul cost.

Mixed precision + PE column tiling: chunks 0..14 (>64 active rows) run
serially in float32r (full fp32 data at 1 cycle/row). The ragged tail is
where most chunks live but few rows are active, so those matmuls are packed
into disjoint column strips of the 128x128 PE array with `tile_position` and
run concurrently: chunks 15..35 (<=64 rows) two per pass, chunks 36..88
(<=32 rows) four per pass, each strip accumulating into its own PSUM band;
a cheap DVE reduction folds the bands at the end. fp32r forbids nonzero PSUM
start partitions, so the tail runs in fp16 (same 1 cycle/row; windows and
white-noise signal are well inside fp16 range). Measured end-to-end relative
error vs the fp32 reference: ~2.3e-4.
"""

import math
import os as _os
from contextlib import ExitStack

import numpy as np

import concourse.bass as bass
import concourse.mybir as mybir
import concourse.tile as tile
from concourse import bacc
from concourse.bass_utils import run_bass_kernel_spmd

# ---- problem constants (hardcoded CQT spec) ----
SR = 22050
N_BINS = 84
BPO = 12
FMIN = 32.7
HOP = 512
B, C, T = 8, 2, 220500
N_CORES = 8

LMAX = 11340           # longest window
F = 409                # frames: 1 + (T - LMAX)//HOP
NCHUNK = 89            # ceil(LMAX/128) contraction chunks
MBLK = 432             # 512-sample blocks of x: (F-1)+(NCHUNK-1)//4+1 = 431, +1 pad
FP = 410               # fp32r needs an even moving free dim; frame 409 is junk
NROWS = 2 * N_BINS     # interleaved (re, im) weight rows
G1ROWS = 128           # group 1 = rows 0..127  (bins 0..63)
G2ROWS = NROWS - G1ROWS  # 40 rows (bins 64..83)
HB_S = 6               # first head chunk with <=96 rows: fp16, carries a
                       # 1-strip partner at PE column position 96
C2S = 14               # first chunk with <=64 active rows (2-way col tiling)
C4S = 36               # first chunk with <=32 active rows (4-way col tiling)
N_WARM = int(_os.environ.get("K_NWARM", "5"))  # PE warm-up matmuls

MM_DT = mybir.dt.float32r   # head matmul dtype (full-rate fp32)
TL_DT = mybir.dt.float16    # tail matmul dtype (col-tiling legal, 1 cyc/row)

_PREP = None
_NC = None
LAST_RESULTS = None


def _params():
    """Host-side constants: compacted weight arrays + chunk geometry."""
    global _PREP
    if _PREP is not None:
        return _PREP

    Q = 1.0 / (2.0 ** (1.0 / BPO) - 1.0)
    freqs = FMIN * 2.0 ** (np.arange(N_BINS, dtype=np.float64) / BPO)
    lengths = np.round(Q * SR / freqs).astype(np.int64)
    assert int(lengths.max()) == LMAX

    t = np.arange(LMAX, dtype=np.float64)
    L = lengths.astype(np.float64)[:, None]
    mask = (t[None, :] < L).astype(np.float64)
    win = 0.5 * (1.0 - np.cos(2.0 * math.pi * t[None, :] / L)) * mask
    phase = (2.0 * math.pi / SR) * freqs[:, None] * t[None, :]
    w_re = (win * np.cos(phase)).astype(np.float32)
    w_im = (-win * np.sin(phase)).astype(np.float32)

    # rows 2k / 2k+1 = re_k / im_k; zero-pad time to NCHUNK*128
    W = np.zeros((NROWS, NCHUNK * 128), dtype=np.float32)
    W[0::2, :LMAX] = w_re
    W[1::2, :LMAX] = w_im
    WT = np.ascontiguousarray(W.T)  # (NCHUNK*128, NROWS)

    n_act = np.array([(lengths > 128 * c).sum() for c in range(NCHUNK)])
    assert n_act[0] == N_BINS and n_act[-1] >= 1
    mG1 = np.minimum(G1ROWS, 2 * n_act).astype(np.int64)
    assert mG1[C2S] <= 64 and mG1[C2S - 1] > 64
    assert mG1[C4S] <= 32 and mG1[C4S - 1] > 32
    G2C = math.ceil(int(lengths[G1ROWS // 2]) / 128)  # chunks needed by bin 64
    mG2 = (2 * n_act[:G2C] - G1ROWS).astype(np.int64)
    assert mG2[0] == G2ROWS and (mG2 > 0).all()

    base = np.zeros(NCHUNK + 1, dtype=np.int64)
    base[1:] = np.cumsum(mG1)
    SG1 = int(base[-1])
    g2base = np.zeros(G2C + 1, dtype=np.int64)
    g2base[1:] = np.cumsum(mG2)
    SG2 = int(g2base[-1])

    wg1 = np.zeros((128, SG1), dtype=np.float32)
    for c in range(NCHUNK):
        wg1[:, base[c]:base[c + 1]] = WT[128 * c:128 * (c + 1), :mG1[c]]
    wg2 = np.zeros((128, SG2), dtype=np.float32)
    for c in range(G2C):
        wg2[:, g2base[c]:g2base[c + 1]] = WT[128 * c:128 * (c + 1),
                                             G1ROWS:G1ROWS + mG2[c]]

    SH = int(base[HB_S])         # fp32r head weight columns (chunks 0..5)
    SHB = int(base[C2S]) - SH    # fp16 paired-head columns (chunks 6..13)
    ST = SG1 - SH - SHB          # fp16 tail columns (chunks 14..88)
    wg1h = np.ascontiguousarray(wg1[:, :SH])
    wg1hb = np.ascontiguousarray(wg1[:, SH:SH + SHB]).astype(np.float16)
    wg1t = np.ascontiguousarray(wg1[:, SH + SHB:]).astype(np.float16)

    _PREP = dict(mG1=mG1, mG2=mG2, G2C=G2C, base=base, g2base=g2base,
                 SH=SH, SHB=SHB, ST=ST, SG2=SG2,
                 wg1h=wg1h, wg1hb=wg1hb, wg1t=wg1t, wg2=wg2)
    return _PREP


def _build_nc(rep=1):
    """Build the per-core Bass module. rep>1 wraps the matmul streams in a
    For_i hardware loop (benchmarking only)."""
    p = _params()
    mG1, mG2, G2C = p["mG1"], p["mG2"], p["G2C"]
    base, g2base = p["base"], p["g2base"]
    SH, SHB, ST, SG2 = p["SH"], p["SHB"], p["ST"], p["SG2"]

    nc = bacc.Bacc(None, target_bir_lowering=False)
    xt_d = nc.dram_tensor("xt", (C, 4, 128, MBLK), MM_DT, kind="ExternalInput")
    xtb_d = nc.dram_tensor("xtb", (C, 4, 128, MBLK), TL_DT,
                           kind="ExternalInput")
    wh_d = nc.dram_tensor("wh", (128, SH), MM_DT, kind="ExternalInput")
    whb_d = nc.dram_tensor("whb", (128, SHB), TL_DT, kind="ExternalInput")
    wt_d = nc.dram_tensor("wt", (128, ST), TL_DT, kind="ExternalInput")
    wg2_d = nc.dram_tensor("wg2", (128, SG2), TL_DT, kind="ExternalInput")
    out_d = nc.dram_tensor("out", (C, NROWS, F), mybir.dt.float32,
                           kind="ExternalOutput")

    with ExitStack() as ctx:
        tc = ctx.enter_context(tile.TileContext(nc))
        xp = ctx.enter_context(tc.tile_pool(name="xp", bufs=1))
        wp = ctx.enter_context(tc.tile_pool(name="wp", bufs=1))
        op = ctx.enter_context(tc.tile_pool(name="op", bufs=1))
        pp = ctx.enter_context(tc.tile_pool(name="pp", bufs=1, space="PSUM"))

        # PSUM: 4 banks per channel (head, G2, 2-way bands, 4-way bands)
        # full-bank width (512) so partition-sliced band APs stay bank-local
        ps1 = {ch: pp.tile([128, 512], mybir.dt.float32, name=f"ps1_{ch}",
                           tag=f"ps1_{ch}") for ch in range(C)}
        ps2 = {ch: pp.tile([128, 512], mybir.dt.float32, name=f"ps2_{ch}",
                           tag=f"ps2_{ch}") for ch in range(C)}
        pt2 = {ch: pp.tile([128, 512], mybir.dt.float32, name=f"pt2_{ch}",
                           tag=f"pt2_{ch}") for ch in range(C)}
        pt4 = {ch: pp.tile([128, 512], mybir.dt.float32, name=f"pt4_{ch}",
                           tag=f"pt4_{ch}") for ch in range(C)}

        # PE warm-up: dummy matmuls on a memset scratch tile bridge the cold
        # pstate while input DMAs run; target ch1's 4-way bank, whose real
        # use starts much later with start=True (pending-zero overwrites).
        warm_sb = xp.tile([128, 128], MM_DT, name="warm_sb", tag="warm_sb")
        nc.vector.memset(warm_sb[:].bitcast(mybir.dt.float32), 0.0)
        for _ in range(N_WARM):
            nc.tensor.matmul(pt4[1][:, 0:128], warm_sb[:, :], warm_sb[:, :],
                             start=True, stop=True, skip_group_check=True)

        # --- SBUF tiles + input DMA plan (two parallel queues) ---
        wh_sb = wp.tile([128, SH], MM_DT, name="wh_sb", tag="wh_sb")
        whb_sb = wp.tile([128, SHB], TL_DT, name="whb_sb", tag="whb_sb")
        wt_sb = wp.tile([128, ST], TL_DT, name="wt_sb", tag="wt_sb")
        wg2_sb = wp.tile([128, SG2], TL_DT, name="wg2_sb", tag="wg2_sb")
        xt_sb = {(ch, rc): xp.tile([128, MBLK], MM_DT, name=f"x_{ch}_{rc}",
                                   tag=f"x_{ch}_{rc}")
                 for ch in range(C) for rc in range(4)}
        xtb_sb = {(ch, rc): xp.tile([128, MBLK], TL_DT, name=f"xb_{ch}_{rc}",
                                    tag=f"xb_{ch}_{rc}")
                  for ch in range(C) for rc in range(4)}

        # gpsimd (SWDGE) stream: weights in consumption order. Split the
        # head weights so the first matmul is gated by a single small block.
        nc.gpsimd.dma_start(wh_sb[:, 0:int(base[1])], wh_d[:, 0:int(base[1])])
        nc.gpsimd.dma_start(wh_sb[:, int(base[1]):SH],
                            wh_d[:, int(base[1]):SH])
        nc.gpsimd.dma_start(whb_sb[:], whb_d[:])
        half_t = (ST // 2) & ~1
        nc.gpsimd.dma_start(wt_sb[:, 0:half_t], wt_d[:, 0:half_t])
        # sync (HWDGE) stream: ch0 signal (fp32r then fp16), G2 weights,
        # second half of the fp16 tail weights (hedges SWDGE bandwidth),
        # then ch1 (needed only from halfway).
        nc.sync.dma_start(xt_sb[0, 0][:], xt_d[0, 0])
        nc.sync.dma_start(wg2_sb[:], wg2_d[:])
        for rc in range(1, 4):
            nc.sync.dma_start(xt_sb[0, rc][:], xt_d[0, rc])
        for rc in (2, 3, 0, 1):  # first 2-way chunks consume rc 2,3 first
            nc.sync.dma_start(xtb_sb[0, rc][:], xtb_d[0, rc])
        nc.sync.dma_start(wt_sb[:, half_t:ST], wt_d[:, half_t:ST])
        for rc in range(4):
            nc.sync.dma_start(xt_sb[1, rc][:], xt_d[1, rc])
        for rc in (2, 3, 0, 1):
            nc.sync.dma_start(xtb_sb[1, rc][:], xtb_d[1, rc])

        # --- fp16 band plan (per channel, identical both channels) ---
        # A band = (psum tile, partition position): an independent strip
        # accumulator. Head chunks 6..13 (fp16, <=96 rows, strips 0-2) each
        # carry a 1-strip pt4@96 partner. G2's three chunks ride in hybrid
        # passes (G2c0@ps2:0 + 2-way partner; G2c1@ps2:0 + G2c2@ps2:64 +
        # two pt4 partners — G2c2's partitions must not overlap G2c0/1's).
        # Chunks are dealt to bands from per-class iterators in emission
        # order, which keeps every band's M sequence descending (so its
        # first, start=True matmul pending-zero-arms all rows it ever uses).
        # `stop` is inert under skip_group_check (and a no-op on HW).
        def g1_chunk(c, cls):
            j, rc = divmod(c, 4)
            offs = {"hb": SH, "tw": SH + SHB, "fw": SH + SHB}
            return dict(m=int(mG1[c]), off=int(base[c]) - offs[cls],
                        j=j, rc=rc, cls=cls)

        def g2_chunk(c):
            j, rc = divmod(c, 4)
            return dict(m=int(mG2[c]), off=int(g2base[c]), j=j, rc=rc,
                        cls="g2")

        srcs = {
            "tw": [g1_chunk(c, "tw") for c in range(C2S, C4S)],    # 22, M<=64
            "fw": [g1_chunk(c, "fw") for c in range(C4S, NCHUNK)], # 53, M<=32
            "g2": [g2_chunk(c) for c in range(G2C)],               # 3
        }
        bands = {
            "hb":   dict(tile="ps1", pos=0,  cls="hb"),
            "2w0":  dict(tile="pt2", pos=0,  cls="tw"),
            "2w64": dict(tile="pt2", pos=64, cls="tw"),
            "g2a":  dict(tile="ps2", pos=0,  cls="g2"),
            "g2b":  dict(tile="ps2", pos=64, cls="g2"),
            "4w0":  dict(tile="pt4", pos=0,  cls="fw"),
            "4w32": dict(tile="pt4", pos=32, cls="fw"),
            "4w64": dict(tile="pt4", pos=64, cls="fw"),
            "4w96": dict(tile="pt4", pos=96, cls="fw"),
        }
        order = (["g2a", "2w64"]                         # hybrid pass A
                 + ["g2a", "g2b", "4w32", "4w96"]        # hybrid pass B
                 + ["2w0", "2w64"] * 11
                 + ["4w0", "4w32", "4w64", "4w96"] * 13)
        band_rows = {}  # band -> rows its first (largest-M) chunk wrote

        def emit_streams():
            for ch in range(C):
                # fp32r head: chunks 0..5 serial, full array width.
                # start=True on chunk 0 (m=128) zero-arms the whole bank
                # region; later ragged prefixes accumulate (group checker
                # can't express this — skipped).
                for c in range(0, C2S):
                    j, rc = divmod(c, 4)
                    m = int(mG1[c])
                    if c < HB_S:
                        wsb_h = wh_sb[:, int(base[c]):int(base[c]) + m]
                        rhs_h = xt_sb[ch, rc][:, j:j + FP]
                    else:  # chunks 6..13 serial too, fp16 operands
                        o = int(base[c]) - SH
                        wsb_h = whb_sb[:, o:o + m]
                        rhs_h = xtb_sb[ch, rc][:, j:j + FP]
                    nc.tensor.matmul(
                        ps1[ch][0:m, 0:FP], wsb_h, rhs_h,
                        start=(c == 0), stop=False,
                        skip_group_check=True)
                # fp16 col-tiled section
                tiles = {"ps1": ps1[ch], "pt2": pt2[ch], "pt4": pt4[ch],
                         "ps2": ps2[ch]}
                wsbs = {"hb": whb_sb, "tw": wt_sb, "fw": wt_sb, "g2": wg2_sb}
                iters = {k: iter(q) for k, q in srcs.items()}
                first = {k: True for k in bands}
                for key in order:
                    b = bands[key]
                    cinfo = next(iters[b["cls"]], None)
                    if cinfo is None:
                        continue
                    m, pos = cinfo["m"], b["pos"]
                    if first[key]:
                        band_rows[key] = m
                    rhs = xtb_sb[ch, cinfo["rc"]][:,
                                                  cinfo["j"]:cinfo["j"] + FP]
                    wsb = wsbs[cinfo["cls"]]
                    off = cinfo["off"]
                    if key == "hb":
                        # 65..96-row chunk: a 3-strip col_grp mask (0x7) is
                        # not a legal ISA combination, so split at row 64 —
                        # strips 0-1 (0x3) + strip 2 (0x4) — both landing in
                        # ps1's own partitions; never re-arm ps1 (chunk 0
                        # started it).
                        nc.tensor.matmul(
                            ps1[ch][0:64, 0:FP], wsb[:, off:off + 64], rhs,
                            start=False, stop=True,
                            tile_position=(0, 0), skip_group_check=True)
                        nc.tensor.matmul(
                            ps1[ch][64:m, 0:FP], wsb[:, off + 64:off + m],
                            rhs, start=False, stop=True,
                            tile_position=(0, 64), skip_group_check=True)
                    else:
                        nc.tensor.matmul(
                            tiles[b["tile"]][pos:pos + m, 0:FP],
                            wsb[:, off:off + m], rhs,
                            start=first[key], stop=True,
                            tile_position=(0, pos),
                            skip_group_check=True)
                    first[key] = False
                for k, it in iters.items():
                    assert next(it, None) is None, f"class {k} not drained"

        if rep > 1:
            with tc.For_i(0, rep, 1) as _i:
                emit_streams()
        else:
            emit_streams()

        # fold the tail bands into the head accumulator and write out
        for ch in range(C):
            o1 = op.tile([128, F], mybir.dt.float32, name=f"o1_{ch}",
                         tag=f"o1_{ch}")
            o2 = op.tile([G2ROWS, F], mybir.dt.float32, name=f"o2_{ch}",
                         tag=f"o2_{ch}")
            # Emit chains in dependency order so the scheduler hoists them
            # into the matmul stream: rows 64..127 of G1 are final when the
            # fp32r head stops; ps2/pt2 bands stop mid-stream; only the four
            # pt4 adds wait for the last matmul.
            nc.vector.tensor_copy(o1[64:128, :], ps1[ch][64:128, 0:F])
            nc.sync.dma_start(out_d[ch, 64:G1ROWS, :], o1[64:128, :])
            nc.vector.tensor_copy(o2[:], ps2[ch][0:G2ROWS, 0:F])
            m2 = int(mG2[2])
            nc.vector.tensor_add(o2[0:m2, :], o2[0:m2, :],
                                 ps2[ch][64:64 + m2, 0:F])
            nc.sync.dma_start(out_d[ch, G1ROWS:NROWS, :], o2[:])
            nc.vector.tensor_copy(o1[0:64, :], ps1[ch][0:64, 0:F])
            for key in ("2w0", "2w64", "4w0", "4w32", "4w64", "4w96"):
                m = int(band_rows[key])  # rows this band ever wrote (<=64)
                pos = bands[key]["pos"]
                tl = {"pt2": pt2[ch], "pt4": pt4[ch]}[bands[key]["tile"]]
                nc.vector.tensor_add(o1[0:m, :], o1[0:m, :],
                                     tl[pos:pos + m, 0:F])
            nc.sync.dma_start(out_d[ch, 0:64, :], o1[0:64, :])
    nc.finalize()
    return nc


def get_nc():
    global _NC
    if _NC is None:
        _NC = _build_nc()
    return _NC


def _pack_x(xb):
    """(C, T) -> (C, 4, 128, MBLK) with xt[ch, rc, r, m] = x[ch, m*512+rc*128+r]."""
    xpad = np.zeros((C, MBLK * 512), dtype=np.float32)
    xpad[:, :T] = xb
    return np.ascontiguousarray(
        xpad.reshape(C, MBLK, 4, 128).transpose(0, 2, 3, 1))


def kernel(x):
    global LAST_RESULTS
    x = np.asarray(x, dtype=np.float32)
    assert x.shape == (B, C, T)
    p = _params()
    in_maps = []
    for b in range(B):
        xt = _pack_x(x[b])
        in_maps.append({"xt": xt, "xtb": xt.astype(np.float16),
                        "wh": p["wg1h"], "whb": p["wg1hb"], "wt": p["wg1t"],
                        "wg2": p["wg2"].astype(np.float16)})
    nc = get_nc()
    res = run_bass_kernel_spmd(nc, in_maps, core_ids=list(range(N_CORES)))
    LAST_RESULTS = res
    out = np.empty((B, C, N_BINS, F, 2), dtype=np.float32)
    for b in range(B):
        raw = np.asarray(res.results[b]["out"])  # (C, NROWS, F)
        out[b] = raw.reshape(C, N_BINS, 2, F).transpose(0, 1, 3, 2)
    return out



# revision 9
# speedup vs baseline: 2.7894x; 2.7894x over previous
"""Trainium2 Bass kernel for a CQT (constant-Q transform) nn.Module.

Reference computation (per batch sample b, channel c):
    out[b, c, k, f, 0] = sum_t x[b, c, f*HOP + t] * w_re[k, t]
    out[b, c, k, f, 1] = sum_t x[b, c, f*HOP + t] * w_im[k, t]
where w_re/w_im are Hann-windowed complex exponentials with per-bin ragged
lengths (longest 11340 samples), HOP=512, 84 bins, 409 frames.

Strategy: data-parallel over the batch (1 sample per NeuronCore, 8 cores).
Per core the PE matmuls put FRAMES on the output partition axis (stationary
operand = a 128-column slice of the resident signal tile) and the 168
interleaved (re,im) bin rows on the moving free axis.  The contraction axis t
is split into 89 chunks of 128; chunk c only involves the 2*n_act[c] rows of
bins whose window is longer than 128*c, so each chunk's matmul moves just
that many rows.  Total moving rows per frame-tile: sum_c 2*n_act[c] = 3208,
vs 89 chunks * 410 frames if frames were the moving axis -- the ragged bin
lengths do the work pruning for us and the stationary (weight-load) side is
pipelined by the PE.

Both channels' frames are concatenated on one virtual frame axis (ch0 blocks
0..430, ch1 blocks 431..861 of the same 512-sample block grid), which lets
7 frame-tiles of 128 cover all 2*409 frames; tile 3 straddles the channel
seam (its middle 22 partitions compute junk that is never written out) and
tile 6 has only 72 live frames.

Chunks are emitted in rc-major rounds (rc = chunk%4 selects which of the 4
resident signal tiles is the stationary source), so the 4 signal tiles and
the weight array (laid out in emission order) can stream in progressively
while the first frame-tile computes.  Dummy warm-up matmuls bridge the PE
p-state ramp during the initial DMA latency and can be sprinkled at round
boundaries to keep the PE busy across any DMA gate.

All operands are fp16 (white-noise signal and |w|<=1 windows are well inside
range; measured end-to-end relative error vs the fp32 reference ~3e-4,
tolerance 2e-2).  PSUM accumulates in fp32: one bank per frame-tile (7) plus
one scratch bank for warm-ups.
"""

import math
import os as _os
from contextlib import ExitStack

import numpy as np

import concourse.bass as bass
import concourse.mybir as mybir
import concourse.tile as tile
from concourse import bacc
from concourse.bass_utils import run_bass_kernel_spmd

# ---- problem constants (hardcoded CQT spec) ----
SR = 22050
N_BINS = 84
BPO = 12
FMIN = 32.7
HOP = 512
B, C, T = 8, 2, 220500
N_CORES = 8

LMAX = 11340            # longest window
F = 409                 # frames: 1 + (T - LMAX)//HOP
NCHUNK = 89             # ceil(LMAX/128) contraction chunks
NROWS = 2 * N_BINS      # interleaved (re, im) weight rows
MBLK = 431              # 512-sample blocks per channel (ceil(220500/512))
MB2 = 2 * MBLK          # concatenated block axis (ch0 | ch1)
NTILE = 7               # frame tiles of 128 over the 840-virtual-frame axis
V0 = [0, 128, 256, 384, 512, 640, 768]
MT = [128, 128, 128, 128, 128, 128, 72]  # live partition count per tile

MM_DT = mybir.dt.float16

N_WARM = int(_os.environ.get("K_NWARM", "20"))   # initial warm-up matmuls
WARM_N = int(_os.environ.get("K_WARMN", "128"))  # their moving size
OUT_DT = mybir.dt.float16  # staging/output dtype (host casts back to f32)

_PREP = None
_NC = None
LAST_RESULTS = None


def _params():
    """Host-side constants: chunk geometry + emission-order weight layout."""
    global _PREP
    if _PREP is not None:
        return _PREP

    Q = 1.0 / (2.0 ** (1.0 / BPO) - 1.0)
    freqs = FMIN * 2.0 ** (np.arange(N_BINS, dtype=np.float64) / BPO)
    lengths = np.round(Q * SR / freqs).astype(np.int64)
    assert int(lengths.max()) == LMAX

    t = np.arange(LMAX, dtype=np.float64)
    L = lengths.astype(np.float64)[:, None]
    mask = (t[None, :] < L).astype(np.float64)
    win = 0.5 * (1.0 - np.cos(2.0 * math.pi * t[None, :] / L)) * mask
    phase = (2.0 * math.pi / SR) * freqs[:, None] * t[None, :]
    w_re = (win * np.cos(phase)).astype(np.float32)
    w_im = (-win * np.sin(phase)).astype(np.float32)

    # rows 2k / 2k+1 = re_k / im_k; zero-pad time to NCHUNK*128
    W = np.zeros((NROWS, NCHUNK * 128), dtype=np.float32)
    W[0::2, :LMAX] = w_re
    W[1::2, :LMAX] = w_im
    WT = np.ascontiguousarray(W.T)  # (NCHUNK*128, NROWS)

    n_act = np.array([(lengths > 128 * c).sum() for c in range(NCHUNK)])
    assert n_act[0] == N_BINS and n_act[-1] >= 1
    mcols = (2 * n_act).astype(np.int64)  # moving rows per chunk

    # emission order: rc-major rounds (all chunks == r mod 4, ascending)
    order = [c for r in range(4) for c in range(r, NCHUNK, 4)]
    assert order[0] == 0

    # weight layout: chunk blocks laid out in emission order, each block
    # column-compacted to its active rows
    woff = np.zeros(NCHUNK, dtype=np.int64)
    off = 0
    for c in order:
        woff[c] = off
        off += mcols[c]
    S = int(off)
    wc = np.zeros((128, S), dtype=np.float16)
    for c in range(NCHUNK):
        wc[:, woff[c]:woff[c] + mcols[c]] = \
            WT[128 * c:128 * (c + 1), :mcols[c]].astype(np.float16)

    _PREP = dict(mcols=mcols, order=order, woff=woff, S=S, wc=wc)
    return _PREP


# ---- DMA schedule (tunable) ----
# weight pieces: (queue, col_lo, col_hi) in wc layout columns, emitted in
# list order per queue.  x pieces: (queue, rc, col_lo, col_hi).
def _dma_plan(p):
    S = p["S"]
    woff, mcols, order = p["woff"], p["mcols"], p["order"]
    # cut weight cols at round boundaries
    r_end = []
    off = 0
    for r in range(4):
        for c in range(r, NCHUNK, 4):
            off += int(mcols[c])
        r_end.append(off)
    # round 0 split into two pieces
    w0_mid = int(woff[24]) if woff[24] > 0 else r_end[0] // 2
    w_pieces = [
        ("sync", 0, w0_mid),
        ("sync", w0_mid, r_end[0]),
        ("gpsimd", r_end[0], r_end[1]),
        ("sync", r_end[1], r_end[2]),
        ("gpsimd", r_end[2], r_end[3]),
    ]
    x_pieces = [
        ("scalar", 0, 0, 288),
        ("scalar", 1, 0, 288),
        ("scalar", 2, 0, 288),
        ("scalar", 3, 0, 288),
        ("scalar", 0, 288, 576),
        ("scalar", 1, 288, 576),
        ("gpsimd", 2, 288, 576),
        ("gpsimd", 3, 288, 576),
        ("scalar", 0, 576, MB2),
        ("scalar", 1, 576, MB2),
        ("scalar", 3, 576, MB2),
        ("gpsimd", 2, 576, MB2),
    ]
    return w_pieces, x_pieces


def _build_nc():
    p = _params()
    mcols, order, woff, S = p["mcols"], p["order"], p["woff"], p["S"]
    w_pieces, x_pieces = _dma_plan(p)

    nc = bacc.Bacc(None, target_bir_lowering=False)
    xt_d = nc.dram_tensor("xt", (4, 128, MB2), MM_DT, kind="ExternalInput")
    wc_d = nc.dram_tensor("wc", (128, S), MM_DT, kind="ExternalInput")
    out_d = nc.dram_tensor("out", (NTILE, 128, NROWS), OUT_DT,
                           kind="ExternalOutput")

    with ExitStack() as ctx:
        tc = ctx.enter_context(tile.TileContext(nc))
        xp = ctx.enter_context(tc.tile_pool(name="xp", bufs=1))
        wp = ctx.enter_context(tc.tile_pool(name="wp", bufs=1))
        op = ctx.enter_context(tc.tile_pool(name="op", bufs=1))
        pp = ctx.enter_context(tc.tile_pool(name="pp", bufs=1, space="PSUM"))

        # PSUM: one full bank per frame-tile + one warm-up scratch bank
        ps = [pp.tile([128, 512], mybir.dt.float32, name=f"ps{t}",
                      tag=f"ps{t}") for t in range(NTILE)]
        pw = pp.tile([128, 512], mybir.dt.float32, name="pw", tag="pw")

        warm_sb = xp.tile([128, max(WARM_N, 128)], MM_DT, name="warm",
                          tag="warm")
        nc.vector.memset(warm_sb[:].bitcast(mybir.dt.float32), 0.0)
        for _ in range(N_WARM):
            nc.tensor.matmul(pw[:, 0:WARM_N], warm_sb[:, 0:128],
                             warm_sb[:, 0:WARM_N],
                             start=True, stop=True, skip_group_check=True)

        # --- SBUF tiles + input DMA streams ---
        xts = [xp.tile([128, MB2], MM_DT, name=f"x{rc}", tag=f"x{rc}")
               for rc in range(4)]
        wcs = wp.tile([128, S], MM_DT, name="wc_sb", tag="wc_sb")

        qs = {"sync": nc.sync, "scalar": nc.scalar, "gpsimd": nc.gpsimd}
        for q, lo, hi in w_pieces:
            qs[q].dma_start(wcs[:, lo:hi], wc_d[:, lo:hi])
        for q, rc, lo, hi in x_pieces:
            qs[q].dma_start(xts[rc][:, lo:hi], xt_d[rc][:, lo:hi])

        # --- matmul streams: one per frame-tile ---
        def emit_chunk(t, c, last_c):
            v0, m = V0[t], MT[t]
            j, rc = divmod(c, 4)
            n = int(mcols[c])
            o = int(woff[c])
            nc.tensor.matmul(
                ps[t][0:m, 0:n],
                xts[rc][:, v0 + j:v0 + j + m],
                wcs[:, o:o + n],
                start=(c == 0), stop=(c == last_c),
                skip_group_check=True)

        ots = [op.tile([128, NROWS], OUT_DT, name=f"o{t}", tag=f"o{t}")
               for t in range(NTILE)]
        for t in range(NTILE - 1):
            for c in order:
                emit_chunk(t, c, order[-1])
            m = MT[t]
            nc.vector.tensor_copy(ots[t][0:m, :], ps[t][0:m, 0:NROWS])
            nc.sync.dma_start(out_d[t, 0:m, :], ots[t][0:m, :])

        # last frame-tile
        t = NTILE - 1
        m = MT[t]
        for c in order:
            emit_chunk(t, c, order[-1])
        nc.vector.tensor_copy(ots[t][0:m, :], ps[t][0:m, 0:NROWS])
        nc.sync.dma_start(out_d[t, 0:m, :], ots[t][0:m, :])
    nc.finalize()
    return nc


def get_nc():
    global _NC
    if _NC is None:
        _NC = _build_nc()
    return _NC


def _pack_x(xb):
    """(C, T) -> (4, 128, MB2) with xt[rc, r, m] = xcat[m*512 + rc*128 + r],
    xcat = [ch0 blocks 0..430 | ch1 blocks 0..430], zero-padded tails."""
    xpad = np.zeros((C, MBLK * 512), dtype=np.float32)
    xpad[:, :T] = xb
    xcat = xpad.reshape(C * MBLK, 512)          # (862, 512) blocks
    xt = xcat.reshape(MB2, 4, 128).transpose(1, 2, 0)
    return np.ascontiguousarray(xt).astype(np.float16)


def kernel(x):
    global LAST_RESULTS
    x = np.asarray(x, dtype=np.float32)
    assert x.shape == (B, C, T)
    p = _params()
    in_maps = []
    for b in range(B):
        in_maps.append({"xt": _pack_x(x[b]), "wc": p["wc"]})
    nc = get_nc()
    res = run_bass_kernel_spmd(nc, in_maps, core_ids=list(range(N_CORES)))
    LAST_RESULTS = res
    out = np.empty((B, C, N_BINS, F, 2), dtype=np.float32)
    for b in range(B):
        raw = np.asarray(res.results[b]["out"])  # (NTILE, 128, NROWS)
        out[b] = _unpack_out(raw)
    return out


def _unpack_out(raw):
    """(NTILE, 128, NROWS) -> (C, N_BINS, F, 2)."""
    raw = np.asarray(raw, dtype=np.float32)
    cat = raw.reshape(NTILE * 128, NROWS)[:V0[-1] + MT[-1]]  # (840, 168)
    o = np.empty((C, N_BINS, F, 2), dtype=np.float32)
    o[0] = cat[0:F].reshape(F, N_BINS, 2).transpose(1, 0, 2)
    o[1] = cat[MBLK:MBLK + F].reshape(F, N_BINS, 2).transpose(1, 0, 2)
    return o


# revision 24
# speedup vs baseline: 3.3933x; 1.2165x over previous
"""Trainium2 Bass kernel for a CQT (constant-Q transform) nn.Module.

Reference computation (per batch sample b, channel c):
    out[b, c, k, f, 0] = sum_t x[b, c, f*HOP + t] * w_re[k, t]
    out[b, c, k, f, 1] = sum_t x[b, c, f*HOP + t] * w_im[k, t]
where w_re/w_im are Hann-windowed complex exponentials with per-bin ragged
lengths (longest 11340 samples), HOP=512, 84 bins, 409 frames.

Strategy: data-parallel over the batch (1 sample per NeuronCore, 8 cores).
Per core the PE matmuls put FRAMES on the output partition axis (stationary
operand = a 128-column slice of the resident signal tile) and the 168
interleaved (re,im) bin rows on the moving free axis.  The contraction axis t
is split into 89 chunks of 128; chunk c only involves the 2*n_act[c] rows of
bins whose window extends past 128*c, so each chunk's matmul moves just that
many rows -- the ragged bin lengths prune the work and the stationary
(weight-load) side is pipelined by the PE.

Both channels' frames are concatenated on one virtual frame axis (ch0 blocks
0..430, ch1 blocks 431..861 of the same 512-sample block grid), which lets
7 frame-tiles of 128 cover all 2*409 frames; tile 3 straddles the channel
seam (its middle 22 partitions compute junk that is never written out) and
tile 6 has only 72 live frames.

Precision split: the Hann window edges (t/L < TH_LO or > TH_HI) carry ~2-5%
of each window's energy but ~1/3 of the matmul rows.  Those column ranges
run as fp8e4 DoubleRow matmuls -- each covers a PAIR of 128-chunks (K=256)
at 0.5 cycles/row, a 4x throughput vs fp16 -- while the energetic window
middles stay fp16.  Measured end-to-end relative error ~2e-3 (tol 2e-2).

Chunks are emitted in rc-major rounds (rc = chunk%4 picks the stationary
signal tile; fp8 pairs ride in rounds 1 and 3 after both their tiles are
resident), so signal tiles and the weight arrays (laid out in emission
order) stream in while the first frame-tile computes.  Dummy warm-up
matmuls bridge the PE p-state ramp during the initial DMA latency; PSUM
accumulates in fp32, one bank per frame-tile plus a warm-up scratch bank.
"""

import math
import os as _os
from contextlib import ExitStack

import ml_dtypes
import numpy as np

import concourse.bass as bass
import concourse.mybir as mybir
import concourse.tile as tile
from concourse import bacc
from concourse.bass_utils import run_bass_kernel_spmd

# ---- problem constants (hardcoded CQT spec) ----
SR = 22050
N_BINS = 84
BPO = 12
FMIN = 32.7
HOP = 512
B, C, T = 8, 2, 220500
N_CORES = 8

LMAX = 11340            # longest window
F = 409                 # frames: 1 + (T - LMAX)//HOP
NCHUNK = 89             # ceil(LMAX/128) contraction chunks
NPAIR = 44              # fp8 DoubleRow chunk pairs (0,1)..(86,87)
NROWS = 2 * N_BINS      # interleaved (re, im) weight rows
MBLK = 431              # 512-sample blocks per channel (ceil(220500/512))
MB2 = 2 * MBLK          # concatenated block axis (ch0 | ch1)
MB2P = 864              # x8 inner width: fp8 dual-row Ldweights requires the
                        # plane stride to be a multiple of 4 (862 -> pad 864)
NTILE = 7               # frame tiles of 128 over the 840-virtual-frame axis
V0 = [0, 128, 256, 384, 512, 640, 768]
MT = [128, 128, 128, 128, 128, 128, 72]  # live partition count per tile

MM_DT = mybir.dt.float16
F8_DT = mybir.dt.float8e4
OUT_DT = mybir.dt.float16  # staging/output dtype (host casts back to f32)
F8_NP = ml_dtypes.float8_e4m3

TH_LO = float(_os.environ.get("K_THLO", "0.25"))  # fp8 window-start region
TH_HI = float(_os.environ.get("K_THHI", "0.75"))  # fp8 window-tail region
N_WARM = int(_os.environ.get("K_NWARM", "3"))    # warm-up matmuls
WARM_N = int(_os.environ.get("K_WARMN", "128"))  # their moving size

_PREP = None
_NC = None
LAST_RESULTS = None


def _params():
    """Host-side constants: chunk geometry, fp8 pair selection, and
    emission-order weight layouts."""
    global _PREP
    if _PREP is not None:
        return _PREP

    Q = 1.0 / (2.0 ** (1.0 / BPO) - 1.0)
    freqs = FMIN * 2.0 ** (np.arange(N_BINS, dtype=np.float64) / BPO)
    lengths = np.round(Q * SR / freqs).astype(np.int64)
    assert int(lengths.max()) == LMAX

    t = np.arange(LMAX, dtype=np.float64)
    L = lengths.astype(np.float64)[:, None]
    mask = (t[None, :] < L).astype(np.float64)
    win = 0.5 * (1.0 - np.cos(2.0 * math.pi * t[None, :] / L)) * mask
    phase = (2.0 * math.pi / SR) * freqs[:, None] * t[None, :]
    w_re = (win * np.cos(phase)).astype(np.float32)
    w_im = (-win * np.sin(phase)).astype(np.float32)

    # rows 2k / 2k+1 = re_k / im_k; zero-pad time to NCHUNK*128
    W = np.zeros((NROWS, NCHUNK * 128), dtype=np.float32)
    W[0::2, :LMAX] = w_re
    W[1::2, :LMAX] = w_im
    WT = np.ascontiguousarray(W.T)  # (NCHUNK*128, NROWS)

    n_act = np.array([(lengths > 128 * c).sum() for c in range(NCHUNK)])
    assert n_act[0] == N_BINS and n_act[-1] >= 1
    mcols = (2 * n_act).astype(np.int64)  # active rows per chunk

    # fp8 pair selection: pair q = chunks (2q, 2q+1), samples [256q, 256q+256)
    # prefix cols [0:p8): bins whose window-start region contains the pair
    # suffix cols [s8:mcols[2q]): bins whose window-tail region contains it
    p8 = np.zeros(NPAIR, dtype=np.int64)
    s8 = np.zeros(NPAIR, dtype=np.int64)
    for q in range(NPAIR):
        lo, hi = 256 * q, 256 * (q + 1)
        p8[q] = 2 * int((lengths >= hi / TH_LO).sum()) if TH_LO > 0 else 0
        n_not_suf = int((lengths * TH_HI > lo).sum())
        s8[q] = 2 * max(n_not_suf, p8[q] // 2)
        s8[q] = min(s8[q], mcols[2 * q])
        p8[q] = min(p8[q], s8[q])
    # pair 0 keeps no fp8 prefix: chunk 0's full-width fp16 matmul must be
    # the stream's first write so its start=True arms the whole PSUM row and
    # every later (narrower) write lands on already-written bytes -- the
    # functional sim asserts uniform pending-zero state per matmul.
    p8[0] = 0
    assert s8[0] == mcols[0]

    def mid_range(c):
        if c == NCHUNK - 1:
            return (0, int(mcols[c]))
        q = c // 2
        lo = int(p8[q])
        hi = int(s8[q]) if c % 2 == 0 else min(int(s8[q]), int(mcols[c]))
        return (lo, max(lo, hi))

    # emission: rc-major rounds of fp16 mids, then all fp8 pairs ascending
    # (their inputs stream in while the mids run)
    order_mid = [c for r in range(4) for c in range(r, NCHUNK, 4)]
    pair_order = list(range(NPAIR))

    # fp16 weight layout: mid blocks in emission order, column-compacted
    woff = {}
    off = 0
    for c in order_mid:
        lo, hi = mid_range(c)
        woff[c] = off
        off += hi - lo
    S = int(off)
    wc = np.zeros((128, S), dtype=np.float16)
    for c in order_mid:
        lo, hi = mid_range(c)
        if hi > lo:
            wc[:, woff[c]:woff[c] + hi - lo] = \
                WT[128 * c:128 * (c + 1), lo:hi].astype(np.float16)

    # fp8 weight layout: pairs in emission order, prefix block then suffix
    # block per pair; plane i = chunk 2q+i
    w8off_pre = np.zeros(NPAIR, dtype=np.int64)
    w8off_suf = np.zeros(NPAIR, dtype=np.int64)
    off = 0
    for q in pair_order:
        w8off_pre[q] = off
        off += int(p8[q])
        w8off_suf[q] = off
        off += int(mcols[2 * q] - s8[q])
    S8 = int(off + (-off) % 4)  # fp8 dual-row plane stride must be 4-aligned
    w8 = np.zeros((128, 2, S8), dtype=np.float32)
    for q in range(NPAIR):
        m0, s, pq = int(mcols[2 * q]), int(s8[q]), int(p8[q])
        for i in range(2):
            blk = WT[128 * (2 * q + i):128 * (2 * q + i + 1), :]
            if pq:
                w8[:, i, w8off_pre[q]:w8off_pre[q] + pq] = blk[:, :pq]
            if m0 > s:
                w8[:, i, w8off_suf[q]:w8off_suf[q] + m0 - s] = blk[:, s:m0]
    w8 = w8.astype(F8_NP)

    _PREP = dict(mcols=mcols, p8=p8, s8=s8, mid_range=mid_range,
                 order_mid=order_mid, pair_order=pair_order,
                 woff=woff, S=S, wc=wc,
                 w8off_pre=w8off_pre, w8off_suf=w8off_suf, S8=S8, w8=w8)
    return _PREP


def _dma_plan(p):
    """(queue, tensor, slice) pieces, in per-queue emission order."""
    S, S8 = p["S"], p["S8"]
    order_mid, mid_range = p["order_mid"], p["mid_range"]
    # fp16 weight column position at each rc-round boundary
    r_end = []
    off = 0
    for r in range(4):
        for c in range(r, NCHUNK, 4):
            lo, hi = mid_range(c)
            off += hi - lo
        r_end.append(off)
    w0_mid = r_end[0] // 2
    w8_mid = S8 // 2
    plan = int(_os.environ.get("K_PLAN", "2"))

    if plan == 0:
        w_pieces = [
            ("sync", "wc", 0, w0_mid),
            ("sync", "wc", w0_mid, r_end[0]),
            ("sync", "wc", r_end[0], r_end[1]),
            ("sync", "wc", r_end[1], r_end[2]),
            ("sync", "wc", r_end[2], r_end[3]),
            ("sync", "w8", 0, w8_mid),
            ("sync", "w8", w8_mid, S8),
        ]
        x_pieces = [
            ("scalar", "xt", 0, 0, 288),
            ("scalar", "xt", 1, 0, 288),
            ("gpsimd", "xt", 2, 0, 288),
            ("gpsimd", "xt", 3, 0, 288),
            ("scalar", "x8", 0, 0, 288),
            ("gpsimd", "x8", 1, 0, 288),
            ("scalar", "xt", 0, 288, MB2),
            ("scalar", "xt", 1, 288, MB2),
            ("gpsimd", "xt", 2, 288, MB2),
            ("gpsimd", "xt", 3, 288, MB2),
            ("scalar", "x8", 0, 288, MB2),
            ("gpsimd", "x8", 1, 288, MB2),
        ]
    elif plan == 1:  # x remainders early, fp8 data late
        w_pieces = [
            ("sync", "wc", 0, w0_mid),
            ("sync", "wc", w0_mid, r_end[0]),
            ("sync", "wc", r_end[0], r_end[1]),
            ("sync", "wc", r_end[1], r_end[2]),
            ("sync", "wc", r_end[2], r_end[3]),
            ("sync", "w8", 0, S8),
        ]
        x_pieces = [
            ("scalar", "xt", 0, 0, 288),
            ("scalar", "xt", 1, 0, 288),
            ("gpsimd", "xt", 2, 0, 288),
            ("gpsimd", "xt", 3, 0, 288),
            ("scalar", "xt", 0, 288, 576),
            ("scalar", "xt", 1, 288, 576),
            ("gpsimd", "xt", 2, 288, 576),
            ("gpsimd", "xt", 3, 288, 576),
            ("scalar", "xt", 0, 576, MB2),
            ("scalar", "xt", 1, 576, MB2),
            ("gpsimd", "xt", 2, 576, MB2),
            ("gpsimd", "xt", 3, 576, MB2),
            ("scalar", "x8", 0, 0, MB2),
            ("gpsimd", "x8", 1, 0, MB2),
        ]
    elif plan == 2:  # interleave x remainders right after windows per queue
        w_pieces = [
            ("sync", "wc", 0, r_end[0]),
            ("sync", "wc", r_end[0], r_end[1]),
            ("sync", "wc", r_end[1], r_end[2]),
            ("sync", "wc", r_end[2], r_end[3]),
            ("sync", "w8", 0, S8),
        ]
        x_pieces = [
            ("scalar", "xt", 0, 0, 288),
            ("scalar", "xt", 1, 0, 288),
            ("gpsimd", "xt", 2, 0, 288),
            ("gpsimd", "xt", 3, 0, 288),
            ("scalar", "xt", 1, 288, 576),
            ("scalar", "xt", 0, 288, 576),
            ("gpsimd", "xt", 3, 288, 576),
            ("gpsimd", "xt", 2, 288, 576),
            ("scalar", "xt", 0, 576, MB2),
            ("scalar", "xt", 1, 576, MB2),
            ("gpsimd", "xt", 2, 576, MB2),
            ("gpsimd", "xt", 3, 576, MB2),
            ("scalar", "x8", 0, 0, MB2),
            ("gpsimd", "x8", 1, 0, MB2),
        ]
    else:  # plan 3: everything big, fewer pieces
        w_pieces = [
            ("sync", "wc", 0, r_end[1]),
            ("sync", "wc", r_end[1], r_end[3]),
            ("sync", "w8", 0, S8),
        ]
        x_pieces = [
            ("scalar", "xt", 0, 0, MB2),
            ("scalar", "xt", 1, 0, MB2),
            ("gpsimd", "xt", 2, 0, MB2),
            ("gpsimd", "xt", 3, 0, MB2),
            ("scalar", "x8", 0, 0, MB2),
            ("gpsimd", "x8", 1, 0, MB2),
        ]
    return w_pieces, x_pieces


def _build_nc():
    p = _params()
    mcols, mid_range = p["mcols"], p["mid_range"]
    woff, S = p["woff"], p["S"]
    p8, s8 = p["p8"], p["s8"]
    w8off_pre, w8off_suf, S8 = p["w8off_pre"], p["w8off_suf"], p["S8"]
    w_pieces, x_pieces = _dma_plan(p)

    nc = bacc.Bacc(None, target_bir_lowering=False)
    xt_d = nc.dram_tensor("xt", (4, 128, MB2), MM_DT, kind="ExternalInput")
    x8_d = nc.dram_tensor("x8", (2, 128, 2, MB2P), F8_DT, kind="ExternalInput")
    wc_d = nc.dram_tensor("wc", (128, S), MM_DT, kind="ExternalInput")
    w8_d = nc.dram_tensor("w8", (128, 2, S8), F8_DT, kind="ExternalInput")
    out_d = nc.dram_tensor("out", (NTILE, 128, NROWS), OUT_DT,
                           kind="ExternalOutput")

    with ExitStack() as ctx:
        tc = ctx.enter_context(tile.TileContext(nc))
        xp = ctx.enter_context(tc.tile_pool(name="xp", bufs=1))
        wp = ctx.enter_context(tc.tile_pool(name="wp", bufs=1))
        op = ctx.enter_context(tc.tile_pool(name="op", bufs=1))
        pp = ctx.enter_context(tc.tile_pool(name="pp", bufs=1, space="PSUM"))

        # PSUM: one full bank per frame-tile + one warm-up scratch bank
        ps = [pp.tile([128, 512], mybir.dt.float32, name=f"ps{t}",
                      tag=f"ps{t}") for t in range(NTILE)]
        pw = pp.tile([128, 512], mybir.dt.float32, name="pw", tag="pw")

        warm_sb = xp.tile([128, max(WARM_N, 128)], MM_DT, name="warm",
                          tag="warm")
        nc.vector.memset(warm_sb[:].bitcast(mybir.dt.float32), 0.0)
        for _ in range(N_WARM):
            nc.tensor.matmul(pw[:, 0:WARM_N], warm_sb[:, 0:128],
                             warm_sb[:, 0:WARM_N],
                             start=True, stop=True, skip_group_check=True)

        # --- SBUF tiles + input DMA streams ---
        xts = [xp.tile([128, MB2], MM_DT, name=f"x{rc}", tag=f"x{rc}")
               for rc in range(4)]
        x8s = [xp.tile([128, 2, MB2P], F8_DT, name=f"x8_{u}", tag=f"x8_{u}")
               for u in range(2)]
        wcs = wp.tile([128, S], MM_DT, name="wc_sb", tag="wc_sb")
        w8s = wp.tile([128, 2, S8], F8_DT, name="w8_sb", tag="w8_sb")

        qs = {"sync": nc.sync, "scalar": nc.scalar, "gpsimd": nc.gpsimd}
        for q, tn, lo, hi in w_pieces:
            if hi <= lo:
                continue
            if tn == "wc":
                qs[q].dma_start(wcs[:, lo:hi], wc_d[:, lo:hi])
            else:
                qs[q].dma_start(w8s[:, :, lo:hi], w8_d[:, :, lo:hi])
        for q, tn, idx, lo, hi in x_pieces:
            if tn == "xt":
                qs[q].dma_start(xts[idx][:, lo:hi], xt_d[idx][:, lo:hi])
            else:
                qs[q].dma_start(x8s[idx][:, :, lo:hi], x8_d[idx][:, :, lo:hi])

        # --- matmul streams: one per frame-tile ---
        DR = mybir.MatmulPerfMode.DoubleRow

        def emit_stream(t):
            v0, m = V0[t], MT[t]
            emits = [("mid", c) for c in p["order_mid"]]
            for q in p["pair_order"]:
                if p8[q] > 0:
                    emits.append(("pre", q))
                if mcols[2 * q] > s8[q]:
                    emits.append(("suf", q))
            emits = [e for e in emits
                     if e[0] != "mid" or mid_range(e[1])[1] > mid_range(e[1])[0]]
            first = True
            for n, (kind, v) in enumerate(emits):
                last = n == len(emits) - 1
                if kind == "mid":
                    c = v
                    lo, hi = mid_range(c)
                    j, rc = divmod(c, 4)
                    nc.tensor.matmul(
                        ps[t][0:m, lo:hi],
                        xts[rc][:, v0 + j:v0 + j + m],
                        wcs[:, woff[c]:woff[c] + hi - lo],
                        start=first, stop=last, skip_group_check=True)
                else:
                    q = v
                    u, jp = q % 2, q // 2
                    if kind == "pre":
                        cols = (0, int(p8[q]))
                        o8 = int(w8off_pre[q])
                    else:
                        cols = (int(s8[q]), int(mcols[2 * q]))
                        o8 = int(w8off_suf[q])
                    n8 = cols[1] - cols[0]
                    nc.tensor.matmul(
                        ps[t][0:m, cols[0]:cols[1]],
                        x8s[u][:, :, v0 + jp:v0 + jp + m],
                        w8s[:, :, o8:o8 + n8],
                        perf_mode=DR,
                        start=first, stop=last, skip_group_check=True)
                first = False

        ots = [op.tile([128, NROWS], OUT_DT, name=f"o{t}", tag=f"o{t}")
               for t in range(NTILE)]
        for t in range(NTILE):
            emit_stream(t)
            m = MT[t]
            nc.vector.tensor_copy(ots[t][0:m, :], ps[t][0:m, 0:NROWS])
            nc.sync.dma_start(out_d[t, 0:m, :], ots[t][0:m, :])
    nc.finalize()
    return nc


def get_nc():
    global _NC
    if _NC is None:
        _NC = _build_nc()
    return _NC


def _pack_x(xb):
    """(C, T) -> fp16 (4, 128, MB2) + fp8 (2, 128, 2, MB2).

    xt[rc, r, m]    = xcat[m*512 + rc*128 + r]
    x8[u, r, i, m]  = xcat[m*512 + u*256 + i*128 + r]
    xcat = [ch0 blocks 0..430 | ch1 blocks 0..430], zero-padded tails."""
    xpad = np.zeros((C, MBLK * 512), dtype=np.float32)
    xpad[:, :T] = xb
    xcat = xpad.reshape(MB2, 512)
    xt = np.ascontiguousarray(
        xcat.reshape(MB2, 4, 128).transpose(1, 2, 0)).astype(np.float16)
    x8 = np.zeros((2, 128, 2, MB2P), dtype=F8_NP)
    x8[:, :, :, :MB2] = np.ascontiguousarray(
        xcat.reshape(MB2, 2, 2, 128).transpose(1, 3, 2, 0)).astype(F8_NP)
    return xt, x8


def kernel(x):
    global LAST_RESULTS
    x = np.asarray(x, dtype=np.float32)
    assert x.shape == (B, C, T)
    p = _params()
    in_maps = []
    for b in range(B):
        xt, x8 = _pack_x(x[b])
        in_maps.append({"xt": xt, "x8": x8, "wc": p["wc"], "w8": p["w8"]})
    nc = get_nc()
    res = run_bass_kernel_spmd(nc, in_maps, core_ids=list(range(N_CORES)))
    LAST_RESULTS = res
    out = np.empty((B, C, N_BINS, F, 2), dtype=np.float32)
    for b in range(B):
        raw = np.asarray(res.results[b]["out"])  # (NTILE, 128, NROWS)
        out[b] = _unpack_out(raw)
    return out


def _unpack_out(raw):
    """(NTILE, 128, NROWS) -> (C, N_BINS, F, 2)."""
    raw = np.asarray(raw, dtype=np.float32)
    cat = raw.reshape(NTILE * 128, NROWS)[:V0[-1] + MT[-1]]  # (840, 168)
    o = np.empty((C, N_BINS, F, 2), dtype=np.float32)
    o[0] = cat[0:F].reshape(F, N_BINS, 2).transpose(1, 0, 2)
    o[1] = cat[MBLK:MBLK + F].reshape(F, N_BINS, 2).transpose(1, 0, 2)
    return o


# revision 25
# speedup vs baseline: 3.5325x; 1.0410x over previous
"""Trainium2 Bass kernel for a CQT (constant-Q transform) nn.Module.

Reference computation (per batch sample b, channel c):
    out[b, c, k, f, 0] = sum_t x[b, c, f*HOP + t] * w_re[k, t]
    out[b, c, k, f, 1] = sum_t x[b, c, f*HOP + t] * w_im[k, t]
where w_re/w_im are Hann-windowed complex exponentials with per-bin ragged
lengths (longest 11340 samples), HOP=512, 84 bins, 409 frames.

Strategy: data-parallel over the batch (1 sample per NeuronCore, 8 cores).
Per core the PE matmuls put FRAMES on the output partition axis (stationary
operand = a 128-column slice of the resident signal tile) and the 168
interleaved (re,im) bin rows on the moving free axis.  The contraction axis t
is split into 89 chunks of 128; chunk c only involves the 2*n_act[c] rows of
bins whose window extends past 128*c, so each chunk's matmul moves just that
many rows -- the ragged bin lengths prune the work and the stationary
(weight-load) side is pipelined by the PE.

Both channels' frames are concatenated on one virtual frame axis (ch0 blocks
0..430, ch1 blocks 431..861 of the same 512-sample block grid), which lets
7 frame-tiles of 128 cover all 2*409 frames; tile 3 straddles the channel
seam (its middle 22 partitions compute junk that is never written out) and
tile 6 has only 72 live frames.

Precision split: the Hann window edges (t/L < TH_LO or > TH_HI) carry ~2-5%
of each window's energy but ~1/3 of the matmul rows.  Those column ranges
run as fp8e4 DoubleRow matmuls -- each covers a PAIR of 128-chunks (K=256)
at 0.5 cycles/row, a 4x throughput vs fp16 -- while the energetic window
middles stay fp16.  Measured end-to-end relative error ~2e-3 (tol 2e-2).

Chunks are emitted in rc-major rounds (rc = chunk%4 picks the stationary
signal tile; fp8 pairs ride in rounds 1 and 3 after both their tiles are
resident), so signal tiles and the weight arrays (laid out in emission
order) stream in while the first frame-tile computes.  Dummy warm-up
matmuls bridge the PE p-state ramp during the initial DMA latency; PSUM
accumulates in fp32, one bank per frame-tile plus a warm-up scratch bank.
"""

import math
import os as _os
from contextlib import ExitStack

import ml_dtypes
import numpy as np

import concourse.bass as bass
import concourse.mybir as mybir
import concourse.tile as tile
from concourse import bacc
from concourse.bass_utils import run_bass_kernel_spmd

# ---- problem constants (hardcoded CQT spec) ----
SR = 22050
N_BINS = 84
BPO = 12
FMIN = 32.7
HOP = 512
B, C, T = 8, 2, 220500
N_CORES = 8

LMAX = 11340            # longest window
F = 409                 # frames: 1 + (T - LMAX)//HOP
NCHUNK = 89             # ceil(LMAX/128) contraction chunks
NPAIR = 44              # fp8 DoubleRow chunk pairs (0,1)..(86,87)
NROWS = 2 * N_BINS      # interleaved (re, im) weight rows
MBLK = 431              # 512-sample blocks per channel (ceil(220500/512))
MB2 = 2 * MBLK          # concatenated block axis (ch0 | ch1)
MB2P = 864              # x8 inner width: fp8 dual-row Ldweights requires the
                        # plane stride to be a multiple of 4 (862 -> pad 864)
NTILE = 7               # frame tiles of 128 over the 840-virtual-frame axis
V0 = [0, 128, 256, 384, 512, 640, 768]
MT = [128, 128, 128, 128, 128, 128, 72]  # live partition count per tile

MM_DT = mybir.dt.float16
F8_DT = mybir.dt.float8e4
OUT_DT = mybir.dt.float16  # staging/output dtype (host casts back to f32)
F8_NP = ml_dtypes.float8_e4m3

TH_LO = float(_os.environ.get("K_THLO", "0.28"))  # fp8 window-start region
TH_HI = float(_os.environ.get("K_THHI", "0.72"))  # fp8 window-tail region
N_WARM = int(_os.environ.get("K_NWARM", "3"))    # warm-up matmuls
WARM_N = int(_os.environ.get("K_WARMN", "128"))  # their moving size

_PREP = None
_NC = None
LAST_RESULTS = None


def _params():
    """Host-side constants: chunk geometry, fp8 pair selection, and
    emission-order weight layouts."""
    global _PREP
    if _PREP is not None:
        return _PREP

    Q = 1.0 / (2.0 ** (1.0 / BPO) - 1.0)
    freqs = FMIN * 2.0 ** (np.arange(N_BINS, dtype=np.float64) / BPO)
    lengths = np.round(Q * SR / freqs).astype(np.int64)
    assert int(lengths.max()) == LMAX

    t = np.arange(LMAX, dtype=np.float64)
    L = lengths.astype(np.float64)[:, None]
    mask = (t[None, :] < L).astype(np.float64)
    win = 0.5 * (1.0 - np.cos(2.0 * math.pi * t[None, :] / L)) * mask
    phase = (2.0 * math.pi / SR) * freqs[:, None] * t[None, :]
    w_re = (win * np.cos(phase)).astype(np.float32)
    w_im = (-win * np.sin(phase)).astype(np.float32)

    # rows 2k / 2k+1 = re_k / im_k; zero-pad time to NCHUNK*128
    W = np.zeros((NROWS, NCHUNK * 128), dtype=np.float32)
    W[0::2, :LMAX] = w_re
    W[1::2, :LMAX] = w_im
    WT = np.ascontiguousarray(W.T)  # (NCHUNK*128, NROWS)

    n_act = np.array([(lengths > 128 * c).sum() for c in range(NCHUNK)])
    assert n_act[0] == N_BINS and n_act[-1] >= 1
    mcols = (2 * n_act).astype(np.int64)  # active rows per chunk

    # fp8 pair selection: pair q = chunks (2q, 2q+1), samples [256q, 256q+256)
    # prefix cols [0:p8): bins whose window-start region contains the pair
    # suffix cols [s8:mcols[2q]): bins whose window-tail region contains it
    p8 = np.zeros(NPAIR, dtype=np.int64)
    s8 = np.zeros(NPAIR, dtype=np.int64)
    for q in range(NPAIR):
        lo, hi = 256 * q, 256 * (q + 1)
        p8[q] = 2 * int((lengths >= hi / TH_LO).sum()) if TH_LO > 0 else 0
        n_not_suf = int((lengths * TH_HI > lo).sum())
        s8[q] = 2 * max(n_not_suf, p8[q] // 2)
        s8[q] = min(s8[q], mcols[2 * q])
        p8[q] = min(p8[q], s8[q])
    # pair 0 keeps no fp8 prefix: chunk 0's full-width fp16 matmul must be
    # the stream's first write so its start=True arms the whole PSUM row and
    # every later (narrower) write lands on already-written bytes -- the
    # functional sim asserts uniform pending-zero state per matmul.
    p8[0] = 0
    assert s8[0] == mcols[0]

    def mid_range(c):
        if c == NCHUNK - 1:
            return (0, int(mcols[c]))
        q = c // 2
        lo = int(p8[q])
        hi = int(s8[q]) if c % 2 == 0 else min(int(s8[q]), int(mcols[c]))
        return (lo, max(lo, hi))

    # emission: rc-major rounds of fp16 mids, then all fp8 pairs ascending
    # (their inputs stream in while the mids run)
    order_mid = [c for r in range(4) for c in range(r, NCHUNK, 4)]
    pair_order = list(range(NPAIR))

    # fp16 weight layout: mid blocks in emission order, column-compacted
    woff = {}
    off = 0
    for c in order_mid:
        lo, hi = mid_range(c)
        woff[c] = off
        off += hi - lo
    S = int(off)
    wc = np.zeros((128, S), dtype=np.float16)
    for c in order_mid:
        lo, hi = mid_range(c)
        if hi > lo:
            wc[:, woff[c]:woff[c] + hi - lo] = \
                WT[128 * c:128 * (c + 1), lo:hi].astype(np.float16)

    # fp8 weight layout: pairs in emission order, prefix block then suffix
    # block per pair; plane i = chunk 2q+i
    w8off_pre = np.zeros(NPAIR, dtype=np.int64)
    w8off_suf = np.zeros(NPAIR, dtype=np.int64)
    off = 0
    for q in pair_order:
        w8off_pre[q] = off
        off += int(p8[q])
        w8off_suf[q] = off
        off += int(mcols[2 * q] - s8[q])
    S8 = int(off + (-off) % 4)  # fp8 dual-row plane stride must be 4-aligned
    w8 = np.zeros((128, 2, S8), dtype=np.float32)
    for q in range(NPAIR):
        m0, s, pq = int(mcols[2 * q]), int(s8[q]), int(p8[q])
        for i in range(2):
            blk = WT[128 * (2 * q + i):128 * (2 * q + i + 1), :]
            if pq:
                w8[:, i, w8off_pre[q]:w8off_pre[q] + pq] = blk[:, :pq]
            if m0 > s:
                w8[:, i, w8off_suf[q]:w8off_suf[q] + m0 - s] = blk[:, s:m0]
    w8 = w8.astype(F8_NP)

    _PREP = dict(mcols=mcols, p8=p8, s8=s8, mid_range=mid_range,
                 order_mid=order_mid, pair_order=pair_order,
                 woff=woff, S=S, wc=wc,
                 w8off_pre=w8off_pre, w8off_suf=w8off_suf, S8=S8, w8=w8)
    return _PREP


def _dma_plan(p):
    """(queue, tensor, slice) pieces, in per-queue emission order."""
    S, S8 = p["S"], p["S8"]
    order_mid, mid_range = p["order_mid"], p["mid_range"]
    # fp16 weight column position at each rc-round boundary
    r_end = []
    off = 0
    for r in range(4):
        for c in range(r, NCHUNK, 4):
            lo, hi = mid_range(c)
            off += hi - lo
        r_end.append(off)
    w0_mid = r_end[0] // 2
    w8_mid = S8 // 2
    plan = int(_os.environ.get("K_PLAN", "2"))

    if plan == 0:
        w_pieces = [
            ("sync", "wc", 0, w0_mid),
            ("sync", "wc", w0_mid, r_end[0]),
            ("sync", "wc", r_end[0], r_end[1]),
            ("sync", "wc", r_end[1], r_end[2]),
            ("sync", "wc", r_end[2], r_end[3]),
            ("sync", "w8", 0, w8_mid),
            ("sync", "w8", w8_mid, S8),
        ]
        x_pieces = [
            ("scalar", "xt", 0, 0, 288),
            ("scalar", "xt", 1, 0, 288),
            ("gpsimd", "xt", 2, 0, 288),
            ("gpsimd", "xt", 3, 0, 288),
            ("scalar", "x8", 0, 0, 288),
            ("gpsimd", "x8", 1, 0, 288),
            ("scalar", "xt", 0, 288, MB2),
            ("scalar", "xt", 1, 288, MB2),
            ("gpsimd", "xt", 2, 288, MB2),
            ("gpsimd", "xt", 3, 288, MB2),
            ("scalar", "x8", 0, 288, MB2),
            ("gpsimd", "x8", 1, 288, MB2),
        ]
    elif plan == 1:  # x remainders early, fp8 data late
        w_pieces = [
            ("sync", "wc", 0, w0_mid),
            ("sync", "wc", w0_mid, r_end[0]),
            ("sync", "wc", r_end[0], r_end[1]),
            ("sync", "wc", r_end[1], r_end[2]),
            ("sync", "wc", r_end[2], r_end[3]),
            ("sync", "w8", 0, S8),
        ]
        x_pieces = [
            ("scalar", "xt", 0, 0, 288),
            ("scalar", "xt", 1, 0, 288),
            ("gpsimd", "xt", 2, 0, 288),
            ("gpsimd", "xt", 3, 0, 288),
            ("scalar", "xt", 0, 288, 576),
            ("scalar", "xt", 1, 288, 576),
            ("gpsimd", "xt", 2, 288, 576),
            ("gpsimd", "xt", 3, 288, 576),
            ("scalar", "xt", 0, 576, MB2),
            ("scalar", "xt", 1, 576, MB2),
            ("gpsimd", "xt", 2, 576, MB2),
            ("gpsimd", "xt", 3, 576, MB2),
            ("scalar", "x8", 0, 0, MB2),
            ("gpsimd", "x8", 1, 0, MB2),
        ]
    elif plan == 2:  # interleave x remainders right after windows per queue
        w_pieces = [
            ("sync", "wc", 0, r_end[0]),
            ("sync", "wc", r_end[0], r_end[1]),
            ("sync", "wc", r_end[1], r_end[2]),
            ("sync", "wc", r_end[2], r_end[3]),
            ("sync", "w8", 0, S8),
        ]
        x_pieces = [
            ("scalar", "xt", 0, 0, 288),
            ("scalar", "xt", 1, 0, 288),
            ("gpsimd", "xt", 2, 0, 288),
            ("gpsimd", "xt", 3, 0, 288),
            ("scalar", "xt", 1, 288, 576),
            ("scalar", "xt", 0, 288, 576),
            ("gpsimd", "xt", 3, 288, 576),
            ("gpsimd", "xt", 2, 288, 576),
            ("scalar", "xt", 0, 576, MB2),
            ("scalar", "xt", 1, 576, MB2),
            ("gpsimd", "xt", 2, 576, MB2),
            ("gpsimd", "xt", 3, 576, MB2),
            ("scalar", "x8", 0, 0, MB2),
            ("gpsimd", "x8", 1, 0, MB2),
        ]
    else:  # plan 3: everything big, fewer pieces
        w_pieces = [
            ("sync", "wc", 0, r_end[1]),
            ("sync", "wc", r_end[1], r_end[3]),
            ("sync", "w8", 0, S8),
        ]
        x_pieces = [
            ("scalar", "xt", 0, 0, MB2),
            ("scalar", "xt", 1, 0, MB2),
            ("gpsimd", "xt", 2, 0, MB2),
            ("gpsimd", "xt", 3, 0, MB2),
            ("scalar", "x8", 0, 0, MB2),
            ("gpsimd", "x8", 1, 0, MB2),
        ]
    return w_pieces, x_pieces


def _build_nc():
    p = _params()
    mcols, mid_range = p["mcols"], p["mid_range"]
    woff, S = p["woff"], p["S"]
    p8, s8 = p["p8"], p["s8"]
    w8off_pre, w8off_suf, S8 = p["w8off_pre"], p["w8off_suf"], p["S8"]
    w_pieces, x_pieces = _dma_plan(p)

    nc = bacc.Bacc(None, target_bir_lowering=False)
    xt_d = nc.dram_tensor("xt", (4, 128, MB2), MM_DT, kind="ExternalInput")
    x8_d = nc.dram_tensor("x8", (2, 128, 2, MB2P), F8_DT, kind="ExternalInput")
    wc_d = nc.dram_tensor("wc", (128, S), MM_DT, kind="ExternalInput")
    w8_d = nc.dram_tensor("w8", (128, 2, S8), F8_DT, kind="ExternalInput")
    out_d = nc.dram_tensor("out", (NTILE, 128, NROWS), OUT_DT,
                           kind="ExternalOutput")

    with ExitStack() as ctx:
        tc = ctx.enter_context(tile.TileContext(nc))
        xp = ctx.enter_context(tc.tile_pool(name="xp", bufs=1))
        wp = ctx.enter_context(tc.tile_pool(name="wp", bufs=1))
        op = ctx.enter_context(tc.tile_pool(name="op", bufs=1))
        pp = ctx.enter_context(tc.tile_pool(name="pp", bufs=1, space="PSUM"))

        # PSUM: one full bank per frame-tile + one warm-up scratch bank
        ps = [pp.tile([128, 512], mybir.dt.float32, name=f"ps{t}",
                      tag=f"ps{t}") for t in range(NTILE)]
        pw = pp.tile([128, 512], mybir.dt.float32, name="pw", tag="pw")

        warm_sb = xp.tile([128, max(WARM_N, 128)], MM_DT, name="warm",
                          tag="warm")
        nc.vector.memset(warm_sb[:].bitcast(mybir.dt.float32), 0.0)
        for _ in range(N_WARM):
            nc.tensor.matmul(pw[:, 0:WARM_N], warm_sb[:, 0:128],
                             warm_sb[:, 0:WARM_N],
                             start=True, stop=True, skip_group_check=True)

        # --- SBUF tiles + input DMA streams ---
        xts = [xp.tile([128, MB2], MM_DT, name=f"x{rc}", tag=f"x{rc}")
               for rc in range(4)]
        x8s = [xp.tile([128, 2, MB2P], F8_DT, name=f"x8_{u}", tag=f"x8_{u}")
               for u in range(2)]
        wcs = wp.tile([128, S], MM_DT, name="wc_sb", tag="wc_sb")
        w8s = wp.tile([128, 2, S8], F8_DT, name="w8_sb", tag="w8_sb")

        qs = {"sync": nc.sync, "scalar": nc.scalar, "gpsimd": nc.gpsimd}
        for q, tn, lo, hi in w_pieces:
            if hi <= lo:
                continue
            if tn == "wc":
                qs[q].dma_start(wcs[:, lo:hi], wc_d[:, lo:hi])
            else:
                qs[q].dma_start(w8s[:, :, lo:hi], w8_d[:, :, lo:hi])
        for q, tn, idx, lo, hi in x_pieces:
            if tn == "xt":
                qs[q].dma_start(xts[idx][:, lo:hi], xt_d[idx][:, lo:hi])
            else:
                qs[q].dma_start(x8s[idx][:, :, lo:hi], x8_d[idx][:, :, lo:hi])

        # --- matmul streams: one per frame-tile ---
        DR = mybir.MatmulPerfMode.DoubleRow

        def emit_stream(t):
            v0, m = V0[t], MT[t]
            emits = [("mid", c) for c in p["order_mid"]]
            for q in p["pair_order"]:
                if p8[q] > 0:
                    emits.append(("pre", q))
                if mcols[2 * q] > s8[q]:
                    emits.append(("suf", q))
            emits = [e for e in emits
                     if e[0] != "mid" or mid_range(e[1])[1] > mid_range(e[1])[0]]
            first = True
            for n, (kind, v) in enumerate(emits):
                last = n == len(emits) - 1
                if kind == "mid":
                    c = v
                    lo, hi = mid_range(c)
                    j, rc = divmod(c, 4)
                    nc.tensor.matmul(
                        ps[t][0:m, lo:hi],
                        xts[rc][:, v0 + j:v0 + j + m],
                        wcs[:, woff[c]:woff[c] + hi - lo],
                        start=first, stop=last, skip_group_check=True)
                else:
                    q = v
                    u, jp = q % 2, q // 2
                    if kind == "pre":
                        cols = (0, int(p8[q]))
                        o8 = int(w8off_pre[q])
                    else:
                        cols = (int(s8[q]), int(mcols[2 * q]))
                        o8 = int(w8off_suf[q])
                    n8 = cols[1] - cols[0]
                    nc.tensor.matmul(
                        ps[t][0:m, cols[0]:cols[1]],
                        x8s[u][:, :, v0 + jp:v0 + jp + m],
                        w8s[:, :, o8:o8 + n8],
                        perf_mode=DR,
                        start=first, stop=last, skip_group_check=True)
                first = False

        ots = [op.tile([128, NROWS], OUT_DT, name=f"o{t}", tag=f"o{t}")
               for t in range(NTILE)]
        for t in range(NTILE):
            emit_stream(t)
            m = MT[t]
            nc.vector.tensor_copy(ots[t][0:m, :], ps[t][0:m, 0:NROWS])
            nc.sync.dma_start(out_d[t, 0:m, :], ots[t][0:m, :])
    nc.finalize()
    return nc


def get_nc():
    global _NC
    if _NC is None:
        _NC = _build_nc()
    return _NC


def _pack_x(xb):
    """(C, T) -> fp16 (4, 128, MB2) + fp8 (2, 128, 2, MB2).

    xt[rc, r, m]    = xcat[m*512 + rc*128 + r]
    x8[u, r, i, m]  = xcat[m*512 + u*256 + i*128 + r]
    xcat = [ch0 blocks 0..430 | ch1 blocks 0..430], zero-padded tails."""
    xpad = np.zeros((C, MBLK * 512), dtype=np.float32)
    xpad[:, :T] = xb
    xcat = xpad.reshape(MB2, 512)
    xt = np.ascontiguousarray(
        xcat.reshape(MB2, 4, 128).transpose(1, 2, 0)).astype(np.float16)
    x8 = np.zeros((2, 128, 2, MB2P), dtype=F8_NP)
    x8[:, :, :, :MB2] = np.ascontiguousarray(
        xcat.reshape(MB2, 2, 2, 128).transpose(1, 3, 2, 0)).astype(F8_NP)
    return xt, x8


def kernel(x):
    global LAST_RESULTS
    x = np.asarray(x, dtype=np.float32)
    assert x.shape == (B, C, T)
    p = _params()
    in_maps = []
    for b in range(B):
        xt, x8 = _pack_x(x[b])
        in_maps.append({"xt": xt, "x8": x8, "wc": p["wc"], "w8": p["w8"]})
    nc = get_nc()
    res = run_bass_kernel_spmd(nc, in_maps, core_ids=list(range(N_CORES)))
    LAST_RESULTS = res
    out = np.empty((B, C, N_BINS, F, 2), dtype=np.float32)
    for b in range(B):
        raw = np.asarray(res.results[b]["out"])  # (NTILE, 128, NROWS)
        out[b] = _unpack_out(raw)
    return out


def _unpack_out(raw):
    """(NTILE, 128, NROWS) -> (C, N_BINS, F, 2)."""
    raw = np.asarray(raw, dtype=np.float32)
    cat = raw.reshape(NTILE * 128, NROWS)[:V0[-1] + MT[-1]]  # (840, 168)
    o = np.empty((C, N_BINS, F, 2), dtype=np.float32)
    o[0] = cat[0:F].reshape(F, N_BINS, 2).transpose(1, 0, 2)
    o[1] = cat[MBLK:MBLK + F].reshape(F, N_BINS, 2).transpose(1, 0, 2)
    return o


# revision 27
# speedup vs baseline: 3.6069x; 1.0211x over previous
"""Trainium2 Bass kernel for a CQT (constant-Q transform) nn.Module.

Reference computation (per batch sample b, channel c):
    out[b, c, k, f, 0] = sum_t x[b, c, f*HOP + t] * w_re[k, t]
    out[b, c, k, f, 1] = sum_t x[b, c, f*HOP + t] * w_im[k, t]
where w_re/w_im are Hann-windowed complex exponentials with per-bin ragged
lengths (longest 11340 samples), HOP=512, 84 bins, 409 frames.

Strategy: data-parallel over the batch (1 sample per NeuronCore, 8 cores).
Per core the PE matmuls put FRAMES on the output partition axis (stationary
operand = a 128-column slice of the resident signal tile) and the 168
interleaved (re,im) bin rows on the moving free axis.  The contraction axis t
is split into 89 chunks of 128; chunk c only involves the 2*n_act[c] rows of
bins whose window extends past 128*c, so each chunk's matmul moves just that
many rows -- the ragged bin lengths prune the work and the stationary
(weight-load) side is pipelined by the PE.

Both channels' frames are concatenated on one virtual frame axis (ch0 blocks
0..430, ch1 blocks 431..861 of the same 512-sample block grid), which lets
7 frame-tiles of 128 cover all 2*409 frames; tile 3 straddles the channel
seam (its middle 22 partitions compute junk that is never written out) and
tile 6 has only 72 live frames.

Precision split: the Hann window edges (t/L < TH_LO or > TH_HI) carry ~8%
of each window's energy but ~1/3 of the matmul rows.  Those column ranges
run as fp8e4 DoubleRow matmuls -- each covers a PAIR of 128-chunks (K=256)
at 0.5 cycles/row, a 4x throughput vs fp16 -- while the energetic window
middles stay fp16.  The fp8 dual-row Ldweights ISA requires the stationary
operand's plane stride to be a multiple of 4, hence the padded x8 layout.
Measured end-to-end relative error ~1.0e-2 (tolerance 2e-2).

Per stream, fp16 mids are emitted in rc-major rounds (rc = chunk%4 picks
the stationary signal tile), then all fp8 pairs -- so signal tiles and the
weight arrays (laid out in emission order) stream in while the first
frame-tile computes.  The Tile scheduler reorders matmuls across streams
by readiness, so the DMA piece split/ordering and the warm-up count are
tuned empirically against the CoreSim schedule (dummy warm-up matmuls also
bridge the PE p-state ramp during the initial DMA latency; an idle PE gap
before the first real matmul would both re-slow the ramp and shift the
scheduler into a worse semaphore batching).  PSUM accumulates in fp32, one
bank per frame-tile plus a warm-up scratch bank.
"""

import math
import os as _os
from contextlib import ExitStack

import ml_dtypes
import numpy as np

import concourse.bass as bass
import concourse.mybir as mybir
import concourse.tile as tile
from concourse import bacc
from concourse.bass_utils import run_bass_kernel_spmd

# ---- problem constants (hardcoded CQT spec) ----
SR = 22050
N_BINS = 84
BPO = 12
FMIN = 32.7
HOP = 512
B, C, T = 8, 2, 220500
N_CORES = 8

LMAX = 11340            # longest window
F = 409                 # frames: 1 + (T - LMAX)//HOP
NCHUNK = 89             # ceil(LMAX/128) contraction chunks
NPAIR = 44              # fp8 DoubleRow chunk pairs (0,1)..(86,87)
NROWS = 2 * N_BINS      # interleaved (re, im) weight rows
MBLK = 431              # 512-sample blocks per channel (ceil(220500/512))
MB2 = 2 * MBLK          # concatenated block axis (ch0 | ch1)
MB2P = 864              # x8 inner width: fp8 dual-row Ldweights requires the
                        # plane stride to be a multiple of 4 (862 -> pad 864)
NTILE = 7               # frame tiles of 128 over the 840-virtual-frame axis
V0 = [0, 128, 256, 384, 512, 640, 768]
MT = [128, 128, 128, 128, 128, 128, 72]  # live partition count per tile

MM_DT = mybir.dt.float16
F8_DT = mybir.dt.float8e4
OUT_DT = mybir.dt.float16  # staging/output dtype (host casts back to f32)
F8_NP = ml_dtypes.float8_e4m3

TH_LO = float(_os.environ.get("K_THLO", "0.28"))  # fp8 window-start region
TH_HI = float(_os.environ.get("K_THHI", "0.72"))  # fp8 window-tail region
N_WARM = int(_os.environ.get("K_NWARM", "3"))    # warm-up matmuls
WARM_N = int(_os.environ.get("K_WARMN", "112"))  # their moving size

_PREP = None
_NC = None
LAST_RESULTS = None


def _params():
    """Host-side constants: chunk geometry, fp8 pair selection, and
    emission-order weight layouts."""
    global _PREP
    if _PREP is not None:
        return _PREP

    Q = 1.0 / (2.0 ** (1.0 / BPO) - 1.0)
    freqs = FMIN * 2.0 ** (np.arange(N_BINS, dtype=np.float64) / BPO)
    lengths = np.round(Q * SR / freqs).astype(np.int64)
    assert int(lengths.max()) == LMAX

    t = np.arange(LMAX, dtype=np.float64)
    L = lengths.astype(np.float64)[:, None]
    mask = (t[None, :] < L).astype(np.float64)
    win = 0.5 * (1.0 - np.cos(2.0 * math.pi * t[None, :] / L)) * mask
    phase = (2.0 * math.pi / SR) * freqs[:, None] * t[None, :]
    w_re = (win * np.cos(phase)).astype(np.float32)
    w_im = (-win * np.sin(phase)).astype(np.float32)

    # rows 2k / 2k+1 = re_k / im_k; zero-pad time to NCHUNK*128
    W = np.zeros((NROWS, NCHUNK * 128), dtype=np.float32)
    W[0::2, :LMAX] = w_re
    W[1::2, :LMAX] = w_im
    WT = np.ascontiguousarray(W.T)  # (NCHUNK*128, NROWS)

    n_act = np.array([(lengths > 128 * c).sum() for c in range(NCHUNK)])
    assert n_act[0] == N_BINS and n_act[-1] >= 1
    mcols = (2 * n_act).astype(np.int64)  # active rows per chunk

    # fp8 pair selection: pair q = chunks (2q, 2q+1), samples [256q, 256q+256)
    # prefix cols [0:p8): bins whose window-start region contains the pair
    # suffix cols [s8:mcols[2q]): bins whose window-tail region contains it
    p8 = np.zeros(NPAIR, dtype=np.int64)
    s8 = np.zeros(NPAIR, dtype=np.int64)
    for q in range(NPAIR):
        lo, hi = 256 * q, 256 * (q + 1)
        p8[q] = 2 * int((lengths >= hi / TH_LO).sum()) if TH_LO > 0 else 0
        n_not_suf = int((lengths * TH_HI > lo).sum())
        s8[q] = 2 * max(n_not_suf, p8[q] // 2)
        s8[q] = min(s8[q], mcols[2 * q])
        p8[q] = min(p8[q], s8[q])
    # pair 0 keeps no fp8 prefix: chunk 0's full-width fp16 matmul must be
    # the stream's first write so its start=True arms the whole PSUM row and
    # every later (narrower) write lands on already-written bytes -- the
    # functional sim asserts uniform pending-zero state per matmul.
    p8[0] = 0
    assert s8[0] == mcols[0]

    def mid_range(c):
        if c == NCHUNK - 1:
            return (0, int(mcols[c]))
        q = c // 2
        lo = int(p8[q])
        hi = int(s8[q]) if c % 2 == 0 else min(int(s8[q]), int(mcols[c]))
        return (lo, max(lo, hi))

    # emission: rc-major rounds of fp16 mids, then all fp8 pairs ascending
    # (their inputs stream in while the mids run)
    order_mid = [c for r in range(4) for c in range(r, NCHUNK, 4)]
    pair_order = list(range(NPAIR))

    # fp16 weight layout: mid blocks in emission order, column-compacted
    woff = {}
    off = 0
    for c in order_mid:
        lo, hi = mid_range(c)
        woff[c] = off
        off += hi - lo
    S = int(off)
    wc = np.zeros((128, S), dtype=np.float16)
    for c in order_mid:
        lo, hi = mid_range(c)
        if hi > lo:
            wc[:, woff[c]:woff[c] + hi - lo] = \
                WT[128 * c:128 * (c + 1), lo:hi].astype(np.float16)

    # fp8 weight layout: pairs in emission order, prefix block then suffix
    # block per pair; plane i = chunk 2q+i
    w8off_pre = np.zeros(NPAIR, dtype=np.int64)
    w8off_suf = np.zeros(NPAIR, dtype=np.int64)
    off = 0
    for q in pair_order:
        w8off_pre[q] = off
        off += int(p8[q])
        w8off_suf[q] = off
        off += int(mcols[2 * q] - s8[q])
    S8 = int(off + (-off) % 4)  # fp8 dual-row plane stride must be 4-aligned
    w8 = np.zeros((128, 2, S8), dtype=np.float32)
    for q in range(NPAIR):
        m0, s, pq = int(mcols[2 * q]), int(s8[q]), int(p8[q])
        for i in range(2):
            blk = WT[128 * (2 * q + i):128 * (2 * q + i + 1), :]
            if pq:
                w8[:, i, w8off_pre[q]:w8off_pre[q] + pq] = blk[:, :pq]
            if m0 > s:
                w8[:, i, w8off_suf[q]:w8off_suf[q] + m0 - s] = blk[:, s:m0]
    w8 = w8.astype(F8_NP)

    _PREP = dict(mcols=mcols, p8=p8, s8=s8, mid_range=mid_range,
                 order_mid=order_mid, pair_order=pair_order,
                 woff=woff, S=S, wc=wc,
                 w8off_pre=w8off_pre, w8off_suf=w8off_suf, S8=S8, w8=w8)
    return _PREP


def _dma_plan(p):
    """(queue, tensor, slice) pieces, in per-queue emission order."""
    S, S8 = p["S"], p["S8"]
    order_mid, mid_range = p["order_mid"], p["mid_range"]
    # fp16 weight column position at each rc-round boundary
    r_end = []
    off = 0
    for r in range(4):
        for c in range(r, NCHUNK, 4):
            lo, hi = mid_range(c)
            off += hi - lo
        r_end.append(off)
    w0_mid = r_end[0] // 2
    w8_mid = S8 // 2
    plan = int(_os.environ.get("K_PLAN", "2"))

    if plan == 0:
        w_pieces = [
            ("sync", "wc", 0, w0_mid),
            ("sync", "wc", w0_mid, r_end[0]),
            ("sync", "wc", r_end[0], r_end[1]),
            ("sync", "wc", r_end[1], r_end[2]),
            ("sync", "wc", r_end[2], r_end[3]),
            ("sync", "w8", 0, w8_mid),
            ("sync", "w8", w8_mid, S8),
        ]
        x_pieces = [
            ("scalar", "xt", 0, 0, 288),
            ("scalar", "xt", 1, 0, 288),
            ("gpsimd", "xt", 2, 0, 288),
            ("gpsimd", "xt", 3, 0, 288),
            ("scalar", "x8", 0, 0, 288),
            ("gpsimd", "x8", 1, 0, 288),
            ("scalar", "xt", 0, 288, MB2),
            ("scalar", "xt", 1, 288, MB2),
            ("gpsimd", "xt", 2, 288, MB2),
            ("gpsimd", "xt", 3, 288, MB2),
            ("scalar", "x8", 0, 288, MB2),
            ("gpsimd", "x8", 1, 288, MB2),
        ]
    elif plan == 1:  # x remainders early, fp8 data late
        w_pieces = [
            ("sync", "wc", 0, w0_mid),
            ("sync", "wc", w0_mid, r_end[0]),
            ("sync", "wc", r_end[0], r_end[1]),
            ("sync", "wc", r_end[1], r_end[2]),
            ("sync", "wc", r_end[2], r_end[3]),
            ("sync", "w8", 0, S8),
        ]
        x_pieces = [
            ("scalar", "xt", 0, 0, 288),
            ("scalar", "xt", 1, 0, 288),
            ("gpsimd", "xt", 2, 0, 288),
            ("gpsimd", "xt", 3, 0, 288),
            ("scalar", "xt", 0, 288, 576),
            ("scalar", "xt", 1, 288, 576),
            ("gpsimd", "xt", 2, 288, 576),
            ("gpsimd", "xt", 3, 288, 576),
            ("scalar", "xt", 0, 576, MB2),
            ("scalar", "xt", 1, 576, MB2),
            ("gpsimd", "xt", 2, 576, MB2),
            ("gpsimd", "xt", 3, 576, MB2),
            ("scalar", "x8", 0, 0, MB2),
            ("gpsimd", "x8", 1, 0, MB2),
        ]
    elif plan == 2:  # interleave x remainders right after windows per queue
        w_pieces = [
            ("sync", "wc", 0, r_end[0]),
            ("sync", "wc", r_end[0], r_end[1]),
            ("sync", "wc", r_end[1], r_end[2]),
            ("sync", "wc", r_end[2], r_end[3]),
            ("sync", "w8", 0, S8),
        ]
        x_pieces = [
            ("scalar", "xt", 0, 0, 288),
            ("scalar", "xt", 1, 0, 288),
            ("gpsimd", "xt", 2, 0, 288),
            ("gpsimd", "xt", 3, 0, 288),
            ("scalar", "xt", 1, 288, 576),
            ("scalar", "xt", 0, 288, 576),
            ("gpsimd", "xt", 3, 288, 576),
            ("gpsimd", "xt", 2, 288, 576),
            ("scalar", "xt", 0, 576, MB2),
            ("scalar", "xt", 1, 576, MB2),
            ("gpsimd", "xt", 2, 576, MB2),
            ("gpsimd", "xt", 3, 576, MB2),
            ("scalar", "x8", 0, 0, MB2),
            ("gpsimd", "x8", 1, 0, MB2),
        ]
    else:  # plan 3: everything big, fewer pieces
        w_pieces = [
            ("sync", "wc", 0, r_end[1]),
            ("sync", "wc", r_end[1], r_end[3]),
            ("sync", "w8", 0, S8),
        ]
        x_pieces = [
            ("scalar", "xt", 0, 0, MB2),
            ("scalar", "xt", 1, 0, MB2),
            ("gpsimd", "xt", 2, 0, MB2),
            ("gpsimd", "xt", 3, 0, MB2),
            ("scalar", "x8", 0, 0, MB2),
            ("gpsimd", "x8", 1, 0, MB2),
        ]
    return w_pieces, x_pieces


def _build_nc():
    p = _params()
    mcols, mid_range = p["mcols"], p["mid_range"]
    woff, S = p["woff"], p["S"]
    p8, s8 = p["p8"], p["s8"]
    w8off_pre, w8off_suf, S8 = p["w8off_pre"], p["w8off_suf"], p["S8"]
    w_pieces, x_pieces = _dma_plan(p)

    nc = bacc.Bacc(None, target_bir_lowering=False)
    xt_d = nc.dram_tensor("xt", (4, 128, MB2), MM_DT, kind="ExternalInput")
    x8_d = nc.dram_tensor("x8", (2, 128, 2, MB2P), F8_DT, kind="ExternalInput")
    wc_d = nc.dram_tensor("wc", (128, S), MM_DT, kind="ExternalInput")
    w8_d = nc.dram_tensor("w8", (128, 2, S8), F8_DT, kind="ExternalInput")
    out_d = nc.dram_tensor("out", (NTILE, 128, NROWS), OUT_DT,
                           kind="ExternalOutput")

    with ExitStack() as ctx:
        tc = ctx.enter_context(tile.TileContext(nc))
        xp = ctx.enter_context(tc.tile_pool(name="xp", bufs=1))
        wp = ctx.enter_context(tc.tile_pool(name="wp", bufs=1))
        op = ctx.enter_context(tc.tile_pool(name="op", bufs=1))
        pp = ctx.enter_context(tc.tile_pool(name="pp", bufs=1, space="PSUM"))

        # PSUM: one full bank per frame-tile + one warm-up scratch bank
        ps = [pp.tile([128, 512], mybir.dt.float32, name=f"ps{t}",
                      tag=f"ps{t}") for t in range(NTILE)]
        pw = pp.tile([128, 512], mybir.dt.float32, name="pw", tag="pw")

        warm_sb = xp.tile([128, max(WARM_N, 128)], MM_DT, name="warm",
                          tag="warm")
        nc.vector.memset(warm_sb[:].bitcast(mybir.dt.float32), 0.0)
        for _ in range(N_WARM):
            nc.tensor.matmul(pw[:, 0:WARM_N], warm_sb[:, 0:128],
                             warm_sb[:, 0:WARM_N],
                             start=True, stop=True, skip_group_check=True)

        # --- SBUF tiles + input DMA streams ---
        xts = [xp.tile([128, MB2], MM_DT, name=f"x{rc}", tag=f"x{rc}")
               for rc in range(4)]
        x8s = [xp.tile([128, 2, MB2P], F8_DT, name=f"x8_{u}", tag=f"x8_{u}")
               for u in range(2)]
        wcs = wp.tile([128, S], MM_DT, name="wc_sb", tag="wc_sb")
        w8s = wp.tile([128, 2, S8], F8_DT, name="w8_sb", tag="w8_sb")

        qs = {"sync": nc.sync, "scalar": nc.scalar, "gpsimd": nc.gpsimd}
        for q, tn, lo, hi in w_pieces:
            if hi <= lo:
                continue
            if tn == "wc":
                qs[q].dma_start(wcs[:, lo:hi], wc_d[:, lo:hi])
            else:
                qs[q].dma_start(w8s[:, :, lo:hi], w8_d[:, :, lo:hi])
        for q, tn, idx, lo, hi in x_pieces:
            if tn == "xt":
                qs[q].dma_start(xts[idx][:, lo:hi], xt_d[idx][:, lo:hi])
            else:
                qs[q].dma_start(x8s[idx][:, :, lo:hi], x8_d[idx][:, :, lo:hi])

        # --- matmul streams: one per frame-tile ---
        DR = mybir.MatmulPerfMode.DoubleRow

        def emit_stream(t):
            v0, m = V0[t], MT[t]
            emits = [("mid", c) for c in p["order_mid"]]
            for q in p["pair_order"]:
                if p8[q] > 0:
                    emits.append(("pre", q))
                if mcols[2 * q] > s8[q]:
                    emits.append(("suf", q))
            emits = [e for e in emits
                     if e[0] != "mid" or mid_range(e[1])[1] > mid_range(e[1])[0]]
            first = True
            for n, (kind, v) in enumerate(emits):
                last = n == len(emits) - 1
                if kind == "mid":
                    c = v
                    lo, hi = mid_range(c)
                    j, rc = divmod(c, 4)
                    nc.tensor.matmul(
                        ps[t][0:m, lo:hi],
                        xts[rc][:, v0 + j:v0 + j + m],
                        wcs[:, woff[c]:woff[c] + hi - lo],
                        start=first, stop=last, skip_group_check=True)
                else:
                    q = v
                    u, jp = q % 2, q // 2
                    if kind == "pre":
                        cols = (0, int(p8[q]))
                        o8 = int(w8off_pre[q])
                    else:
                        cols = (int(s8[q]), int(mcols[2 * q]))
                        o8 = int(w8off_suf[q])
                    n8 = cols[1] - cols[0]
                    nc.tensor.matmul(
                        ps[t][0:m, cols[0]:cols[1]],
                        x8s[u][:, :, v0 + jp:v0 + jp + m],
                        w8s[:, :, o8:o8 + n8],
                        perf_mode=DR,
                        start=first, stop=last, skip_group_check=True)
                first = False

        ots = [op.tile([128, NROWS], OUT_DT, name=f"o{t}", tag=f"o{t}")
               for t in range(NTILE)]
        for t in range(NTILE):
            emit_stream(t)
            m = MT[t]
            nc.vector.tensor_copy(ots[t][0:m, :], ps[t][0:m, 0:NROWS])
            nc.sync.dma_start(out_d[t, 0:m, :], ots[t][0:m, :])
    nc.finalize()
    return nc


def get_nc():
    global _NC
    if _NC is None:
        _NC = _build_nc()
    return _NC


def _pack_x(xb):
    """(C, T) -> fp16 (4, 128, MB2) + fp8 (2, 128, 2, MB2).

    xt[rc, r, m]    = xcat[m*512 + rc*128 + r]
    x8[u, r, i, m]  = xcat[m*512 + u*256 + i*128 + r]
    xcat = [ch0 blocks 0..430 | ch1 blocks 0..430], zero-padded tails."""
    xpad = np.zeros((C, MBLK * 512), dtype=np.float32)
    xpad[:, :T] = xb
    xcat = xpad.reshape(MB2, 512)
    xt = np.ascontiguousarray(
        xcat.reshape(MB2, 4, 128).transpose(1, 2, 0)).astype(np.float16)
    x8 = np.zeros((2, 128, 2, MB2P), dtype=F8_NP)
    x8[:, :, :, :MB2] = np.ascontiguousarray(
        xcat.reshape(MB2, 2, 2, 128).transpose(1, 3, 2, 0)).astype(F8_NP)
    return xt, x8


def kernel(x):
    global LAST_RESULTS
    x = np.asarray(x, dtype=np.float32)
    assert x.shape == (B, C, T)
    p = _params()
    in_maps = []
    for b in range(B):
        xt, x8 = _pack_x(x[b])
        in_maps.append({"xt": xt, "x8": x8, "wc": p["wc"], "w8": p["w8"]})
    nc = get_nc()
    res = run_bass_kernel_spmd(nc, in_maps, core_ids=list(range(N_CORES)))
    LAST_RESULTS = res
    out = np.empty((B, C, N_BINS, F, 2), dtype=np.float32)
    for b in range(B):
        raw = np.asarray(res.results[b]["out"])  # (NTILE, 128, NROWS)
        out[b] = _unpack_out(raw)
    return out


def _unpack_out(raw):
    """(NTILE, 128, NROWS) -> (C, N_BINS, F, 2)."""
    raw = np.asarray(raw, dtype=np.float32)
    cat = raw.reshape(NTILE * 128, NROWS)[:V0[-1] + MT[-1]]  # (840, 168)
    o = np.empty((C, N_BINS, F, 2), dtype=np.float32)
    o[0] = cat[0:F].reshape(F, N_BINS, 2).transpose(1, 0, 2)
    o[1] = cat[MBLK:MBLK + F].reshape(F, N_BINS, 2).transpose(1, 0, 2)
    return o


# revision 31
# speedup vs baseline: 3.6678x; 1.0169x over previous
"""Trainium2 Bass kernel for a CQT (constant-Q transform) nn.Module.

Reference computation (per batch sample b, channel c):
    out[b, c, k, f, 0] = sum_t x[b, c, f*HOP + t] * w_re[k, t]
    out[b, c, k, f, 1] = sum_t x[b, c, f*HOP + t] * w_im[k, t]
where w_re/w_im are Hann-windowed complex exponentials with per-bin ragged
lengths (longest 11340 samples), HOP=512, 84 bins, 409 frames.

Strategy: data-parallel over the batch (1 sample per NeuronCore, 8 cores).
Per core the PE matmuls put FRAMES on the output partition axis (stationary
operand = a 128-column slice of the resident signal tile) and the 168
interleaved (re,im) bin rows on the moving free axis.  The contraction axis t
is split into 89 chunks of 128; chunk c only involves the 2*n_act[c] rows of
bins whose window extends past 128*c, so each chunk's matmul moves just that
many rows -- the ragged bin lengths prune the work and the stationary
(weight-load) side is pipelined by the PE.

Both channels' frames are concatenated on one virtual frame axis (ch0 blocks
0..430, ch1 blocks 431..861 of the same 512-sample block grid), which lets
7 frame-tiles of 128 cover all 2*409 frames; tile 3 straddles the channel
seam (its middle 22 partitions compute junk that is never written out) and
tile 6 has only 72 live frames.

Precision split: the Hann window edges (t/L < TH_LO or > TH_HI) carry ~8%
of each window's energy but ~1/3 of the matmul rows.  Those column ranges
run as fp8e4 DoubleRow matmuls -- each covers a PAIR of 128-chunks (K=256)
at 0.5 cycles/row, a 4x throughput vs fp16 -- while the energetic window
middles stay fp16.  The fp8 dual-row Ldweights ISA requires the stationary
operand's plane stride to be a multiple of 4, hence the padded x8 layout.
Measured end-to-end relative error ~1.0e-2 (tolerance 2e-2).

Per stream, fp16 mids are emitted in rc-major rounds (rc = chunk%4 picks
the stationary signal tile), then all fp8 pairs -- so signal tiles and the
weight arrays (laid out in emission order) stream in while the first
frame-tile computes.  The Tile scheduler reorders matmuls across streams
by readiness, so the DMA piece split/ordering and the warm-up count are
tuned empirically against the CoreSim schedule (dummy warm-up matmuls also
bridge the PE p-state ramp during the initial DMA latency; an idle PE gap
before the first real matmul would both re-slow the ramp and shift the
scheduler into a worse semaphore batching).  PSUM accumulates in fp32, one
bank per frame-tile plus a warm-up scratch bank.
"""

import math
import os as _os
from contextlib import ExitStack

import ml_dtypes
import numpy as np

import concourse.bass as bass
import concourse.mybir as mybir
import concourse.tile as tile
from concourse import bacc
from concourse.bass_utils import run_bass_kernel_spmd

# ---- problem constants (hardcoded CQT spec) ----
SR = 22050
N_BINS = 84
BPO = 12
FMIN = 32.7
HOP = 512
B, C, T = 8, 2, 220500
N_CORES = 8

LMAX = 11340            # longest window
F = 409                 # frames: 1 + (T - LMAX)//HOP
NCHUNK = 89             # ceil(LMAX/128) contraction chunks
NPAIR = 44              # fp8 DoubleRow chunk pairs (0,1)..(86,87)
NROWS = 2 * N_BINS      # interleaved (re, im) weight rows
MBLK = 431              # 512-sample blocks per channel (ceil(220500/512))
MB2 = 2 * MBLK          # concatenated block axis (ch0 | ch1)
MB2P = 864              # x8 inner width: fp8 dual-row Ldweights requires the
                        # plane stride to be a multiple of 4 (862 -> pad 864)
NTILE = 7               # frame tiles of 128 over the 840-virtual-frame axis
V0 = [0, 128, 256, 384, 512, 640, 768]
MT = [128, 128, 128, 128, 128, 128, 72]  # live partition count per tile

MM_DT = mybir.dt.float16
F8_DT = mybir.dt.float8e4
OUT_DT = mybir.dt.float16  # staging/output dtype (host casts back to f32)
F8_NP = ml_dtypes.float8_e4m3

TH_LO = float(_os.environ.get("K_THLO", "0.30"))  # fp8 window-start region
TH_HI = float(_os.environ.get("K_THHI", "0.70"))  # fp8 window-tail region
N_WARM = int(_os.environ.get("K_NWARM", "3"))    # warm-up matmuls
WARM_N = int(_os.environ.get("K_WARMN", "112"))  # their moving size

_PREP = None
_NC = None
LAST_RESULTS = None


def _params():
    """Host-side constants: chunk geometry, fp8 pair selection, and
    emission-order weight layouts."""
    global _PREP
    if _PREP is not None:
        return _PREP

    Q = 1.0 / (2.0 ** (1.0 / BPO) - 1.0)
    freqs = FMIN * 2.0 ** (np.arange(N_BINS, dtype=np.float64) / BPO)
    lengths = np.round(Q * SR / freqs).astype(np.int64)
    assert int(lengths.max()) == LMAX

    t = np.arange(LMAX, dtype=np.float64)
    L = lengths.astype(np.float64)[:, None]
    mask = (t[None, :] < L).astype(np.float64)
    win = 0.5 * (1.0 - np.cos(2.0 * math.pi * t[None, :] / L)) * mask
    phase = (2.0 * math.pi / SR) * freqs[:, None] * t[None, :]
    w_re = (win * np.cos(phase)).astype(np.float32)
    w_im = (-win * np.sin(phase)).astype(np.float32)

    # rows 2k / 2k+1 = re_k / im_k; zero-pad time to NCHUNK*128
    W = np.zeros((NROWS, NCHUNK * 128), dtype=np.float32)
    W[0::2, :LMAX] = w_re
    W[1::2, :LMAX] = w_im
    WT = np.ascontiguousarray(W.T)  # (NCHUNK*128, NROWS)

    n_act = np.array([(lengths > 128 * c).sum() for c in range(NCHUNK)])
    assert n_act[0] == N_BINS and n_act[-1] >= 1
    mcols = (2 * n_act).astype(np.int64)  # active rows per chunk

    # fp8 pair selection: pair q = chunks (2q, 2q+1), samples [256q, 256q+256)
    # prefix cols [0:p8): bins whose window-start region contains the pair
    # suffix cols [s8:mcols[2q]): bins whose window-tail region contains it
    p8 = np.zeros(NPAIR, dtype=np.int64)
    s8 = np.zeros(NPAIR, dtype=np.int64)
    for q in range(NPAIR):
        lo, hi = 256 * q, 256 * (q + 1)
        p8[q] = 2 * int((lengths >= hi / TH_LO).sum()) if TH_LO > 0 else 0
        n_not_suf = int((lengths * TH_HI > lo).sum())
        s8[q] = 2 * max(n_not_suf, p8[q] // 2)
        s8[q] = min(s8[q], mcols[2 * q])
        p8[q] = min(p8[q], s8[q])
    # pair 0 keeps no fp8 prefix: chunk 0's full-width fp16 matmul must be
    # the stream's first write so its start=True arms the whole PSUM row and
    # every later (narrower) write lands on already-written bytes -- the
    # functional sim asserts uniform pending-zero state per matmul.
    p8[0] = 0
    assert s8[0] == mcols[0]

    def mid_range(c):
        if c == NCHUNK - 1:
            return (0, int(mcols[c]))
        q = c // 2
        lo = int(p8[q])
        hi = int(s8[q]) if c % 2 == 0 else min(int(s8[q]), int(mcols[c]))
        return (lo, max(lo, hi))

    # emission: rc-major rounds of fp16 mids, then all fp8 pairs ascending
    # (their inputs stream in while the mids run)
    order_mid = [c for r in range(4) for c in range(r, NCHUNK, 4)]
    pair_order = list(range(NPAIR))

    # fp16 weight layout: mid blocks in emission order, column-compacted
    woff = {}
    off = 0
    for c in order_mid:
        lo, hi = mid_range(c)
        woff[c] = off
        off += hi - lo
    S = int(off)
    wc = np.zeros((128, S), dtype=np.float16)
    for c in order_mid:
        lo, hi = mid_range(c)
        if hi > lo:
            wc[:, woff[c]:woff[c] + hi - lo] = \
                WT[128 * c:128 * (c + 1), lo:hi].astype(np.float16)

    # fp8 weight layout: pairs in emission order, prefix block then suffix
    # block per pair; plane i = chunk 2q+i
    w8off_pre = np.zeros(NPAIR, dtype=np.int64)
    w8off_suf = np.zeros(NPAIR, dtype=np.int64)
    off = 0
    for q in pair_order:
        w8off_pre[q] = off
        off += int(p8[q])
        w8off_suf[q] = off
        off += int(mcols[2 * q] - s8[q])
    S8 = int(off + (-off) % 4)  # fp8 dual-row plane stride must be 4-aligned
    w8 = np.zeros((128, 2, S8), dtype=np.float32)
    for q in range(NPAIR):
        m0, s, pq = int(mcols[2 * q]), int(s8[q]), int(p8[q])
        for i in range(2):
            blk = WT[128 * (2 * q + i):128 * (2 * q + i + 1), :]
            if pq:
                w8[:, i, w8off_pre[q]:w8off_pre[q] + pq] = blk[:, :pq]
            if m0 > s:
                w8[:, i, w8off_suf[q]:w8off_suf[q] + m0 - s] = blk[:, s:m0]
    w8 = w8.astype(F8_NP)

    _PREP = dict(mcols=mcols, p8=p8, s8=s8, mid_range=mid_range,
                 order_mid=order_mid, pair_order=pair_order,
                 woff=woff, S=S, wc=wc,
                 w8off_pre=w8off_pre, w8off_suf=w8off_suf, S8=S8, w8=w8)
    return _PREP


def _dma_plan(p):
    """(queue, tensor, slice) pieces, in per-queue emission order."""
    S, S8 = p["S"], p["S8"]
    order_mid, mid_range = p["order_mid"], p["mid_range"]
    # fp16 weight column position at each rc-round boundary
    r_end = []
    off = 0
    for r in range(4):
        for c in range(r, NCHUNK, 4):
            lo, hi = mid_range(c)
            off += hi - lo
        r_end.append(off)
    w0_mid = r_end[0] // 2
    w8_mid = S8 // 2
    plan = int(_os.environ.get("K_PLAN", "5"))

    if plan == 0:
        w_pieces = [
            ("sync", "wc", 0, w0_mid),
            ("sync", "wc", w0_mid, r_end[0]),
            ("sync", "wc", r_end[0], r_end[1]),
            ("sync", "wc", r_end[1], r_end[2]),
            ("sync", "wc", r_end[2], r_end[3]),
            ("sync", "w8", 0, w8_mid),
            ("sync", "w8", w8_mid, S8),
        ]
        x_pieces = [
            ("scalar", "xt", 0, 0, 288),
            ("scalar", "xt", 1, 0, 288),
            ("gpsimd", "xt", 2, 0, 288),
            ("gpsimd", "xt", 3, 0, 288),
            ("scalar", "x8", 0, 0, 288),
            ("gpsimd", "x8", 1, 0, 288),
            ("scalar", "xt", 0, 288, MB2),
            ("scalar", "xt", 1, 288, MB2),
            ("gpsimd", "xt", 2, 288, MB2),
            ("gpsimd", "xt", 3, 288, MB2),
            ("scalar", "x8", 0, 288, MB2),
            ("gpsimd", "x8", 1, 288, MB2),
        ]
    elif plan == 1:  # x remainders early, fp8 data late
        w_pieces = [
            ("sync", "wc", 0, w0_mid),
            ("sync", "wc", w0_mid, r_end[0]),
            ("sync", "wc", r_end[0], r_end[1]),
            ("sync", "wc", r_end[1], r_end[2]),
            ("sync", "wc", r_end[2], r_end[3]),
            ("sync", "w8", 0, S8),
        ]
        x_pieces = [
            ("scalar", "xt", 0, 0, 288),
            ("scalar", "xt", 1, 0, 288),
            ("gpsimd", "xt", 2, 0, 288),
            ("gpsimd", "xt", 3, 0, 288),
            ("scalar", "xt", 0, 288, 576),
            ("scalar", "xt", 1, 288, 576),
            ("gpsimd", "xt", 2, 288, 576),
            ("gpsimd", "xt", 3, 288, 576),
            ("scalar", "xt", 0, 576, MB2),
            ("scalar", "xt", 1, 576, MB2),
            ("gpsimd", "xt", 2, 576, MB2),
            ("gpsimd", "xt", 3, 576, MB2),
            ("scalar", "x8", 0, 0, MB2),
            ("gpsimd", "x8", 1, 0, MB2),
        ]
    elif plan == 2:  # interleave x remainders right after windows per queue
        w_pieces = [
            ("sync", "wc", 0, r_end[0]),
            ("sync", "wc", r_end[0], r_end[1]),
            ("sync", "wc", r_end[1], r_end[2]),
            ("sync", "wc", r_end[2], r_end[3]),
            ("sync", "w8", 0, S8),
        ]
        x_pieces = [
            ("scalar", "xt", 0, 0, 288),
            ("scalar", "xt", 1, 0, 288),
            ("gpsimd", "xt", 2, 0, 288),
            ("gpsimd", "xt", 3, 0, 288),
            ("scalar", "xt", 1, 288, 576),
            ("scalar", "xt", 0, 288, 576),
            ("gpsimd", "xt", 3, 288, 576),
            ("gpsimd", "xt", 2, 288, 576),
            ("scalar", "xt", 0, 576, MB2),
            ("scalar", "xt", 1, 576, MB2),
            ("gpsimd", "xt", 2, 576, MB2),
            ("gpsimd", "xt", 3, 576, MB2),
            ("scalar", "x8", 0, 0, MB2),
            ("gpsimd", "x8", 1, 0, MB2),
        ]
    elif plan == 3:  # everything big, fewer pieces
        w_pieces = [
            ("sync", "wc", 0, r_end[1]),
            ("sync", "wc", r_end[1], r_end[3]),
            ("sync", "w8", 0, S8),
        ]
        x_pieces = [
            ("scalar", "xt", 0, 0, MB2),
            ("scalar", "xt", 1, 0, MB2),
            ("gpsimd", "xt", 2, 0, MB2),
            ("gpsimd", "xt", 3, 0, MB2),
            ("scalar", "x8", 0, 0, MB2),
            ("gpsimd", "x8", 1, 0, MB2),
        ]
    elif plan == 4:  # plan2 with w8 earlier on sync
        w_pieces = [
            ("sync", "wc", 0, r_end[0]),
            ("sync", "wc", r_end[0], r_end[1]),
            ("sync", "w8", 0, w8_mid),
            ("sync", "wc", r_end[1], r_end[2]),
            ("sync", "wc", r_end[2], r_end[3]),
            ("sync", "w8", w8_mid, S8),
        ]
        x_pieces = [
            ("scalar", "xt", 0, 0, 288),
            ("scalar", "xt", 1, 0, 288),
            ("gpsimd", "xt", 2, 0, 288),
            ("gpsimd", "xt", 3, 0, 288),
            ("scalar", "xt", 1, 288, 576),
            ("scalar", "xt", 0, 288, 576),
            ("gpsimd", "xt", 3, 288, 576),
            ("gpsimd", "xt", 2, 288, 576),
            ("scalar", "xt", 0, 576, MB2),
            ("scalar", "xt", 1, 576, MB2),
            ("gpsimd", "xt", 2, 576, MB2),
            ("gpsimd", "xt", 3, 576, MB2),
            ("scalar", "x8", 0, 0, MB2),
            ("gpsimd", "x8", 1, 0, MB2),
        ]
    elif plan == 5:  # plan2 with x8 split windows early
        w_pieces = [
            ("sync", "wc", 0, r_end[0]),
            ("sync", "wc", r_end[0], r_end[1]),
            ("sync", "wc", r_end[1], r_end[2]),
            ("sync", "wc", r_end[2], r_end[3]),
            ("sync", "w8", 0, S8),
        ]
        x_pieces = [
            ("scalar", "xt", 0, 0, 288),
            ("scalar", "xt", 1, 0, 288),
            ("gpsimd", "xt", 2, 0, 288),
            ("gpsimd", "xt", 3, 0, 288),
            ("scalar", "x8", 0, 0, 288),
            ("gpsimd", "x8", 1, 0, 288),
            ("scalar", "xt", 1, 288, 576),
            ("scalar", "xt", 0, 288, 576),
            ("gpsimd", "xt", 3, 288, 576),
            ("gpsimd", "xt", 2, 288, 576),
            ("scalar", "xt", 0, 576, MB2),
            ("scalar", "xt", 1, 576, MB2),
            ("gpsimd", "xt", 2, 576, MB2),
            ("gpsimd", "xt", 3, 576, MB2),
            ("scalar", "x8", 0, 288, MB2),
            ("gpsimd", "x8", 1, 288, MB2),
        ]
    elif plan == 6:  # plan2 with 256-col windows
        w_pieces = [
            ("sync", "wc", 0, r_end[0]),
            ("sync", "wc", r_end[0], r_end[1]),
            ("sync", "wc", r_end[1], r_end[2]),
            ("sync", "wc", r_end[2], r_end[3]),
            ("sync", "w8", 0, S8),
        ]
        x_pieces = [
            ("scalar", "xt", 0, 0, 256),
            ("scalar", "xt", 1, 0, 256),
            ("gpsimd", "xt", 2, 0, 256),
            ("gpsimd", "xt", 3, 0, 256),
            ("scalar", "xt", 1, 256, 576),
            ("scalar", "xt", 0, 256, 576),
            ("gpsimd", "xt", 3, 256, 576),
            ("gpsimd", "xt", 2, 256, 576),
            ("scalar", "xt", 0, 576, MB2),
            ("scalar", "xt", 1, 576, MB2),
            ("gpsimd", "xt", 2, 576, MB2),
            ("gpsimd", "xt", 3, 576, MB2),
            ("scalar", "x8", 0, 0, MB2),
            ("gpsimd", "x8", 1, 0, MB2),
        ]
    else:  # plan 7: plan2 with wc r0 split
        w_pieces = [
            ("sync", "wc", 0, w0_mid),
            ("sync", "wc", w0_mid, r_end[0]),
            ("sync", "wc", r_end[0], r_end[1]),
            ("sync", "wc", r_end[1], r_end[2]),
            ("sync", "wc", r_end[2], r_end[3]),
            ("sync", "w8", 0, S8),
        ]
        x_pieces = [
            ("scalar", "xt", 0, 0, 288),
            ("scalar", "xt", 1, 0, 288),
            ("gpsimd", "xt", 2, 0, 288),
            ("gpsimd", "xt", 3, 0, 288),
            ("scalar", "xt", 1, 288, 576),
            ("scalar", "xt", 0, 288, 576),
            ("gpsimd", "xt", 3, 288, 576),
            ("gpsimd", "xt", 2, 288, 576),
            ("scalar", "xt", 0, 576, MB2),
            ("scalar", "xt", 1, 576, MB2),
            ("gpsimd", "xt", 2, 576, MB2),
            ("gpsimd", "xt", 3, 576, MB2),
            ("scalar", "x8", 0, 0, MB2),
            ("gpsimd", "x8", 1, 0, MB2),
        ]
    return w_pieces, x_pieces


def _build_nc():
    p = _params()
    mcols, mid_range = p["mcols"], p["mid_range"]
    woff, S = p["woff"], p["S"]
    p8, s8 = p["p8"], p["s8"]
    w8off_pre, w8off_suf, S8 = p["w8off_pre"], p["w8off_suf"], p["S8"]
    w_pieces, x_pieces = _dma_plan(p)

    nc = bacc.Bacc(None, target_bir_lowering=False)
    xt_d = nc.dram_tensor("xt", (4, 128, MB2), MM_DT, kind="ExternalInput")
    x8_d = nc.dram_tensor("x8", (2, 128, 2, MB2P), F8_DT, kind="ExternalInput")
    wc_d = nc.dram_tensor("wc", (128, S), MM_DT, kind="ExternalInput")
    w8_d = nc.dram_tensor("w8", (128, 2, S8), F8_DT, kind="ExternalInput")
    out_d = nc.dram_tensor("out", (NTILE, 128, NROWS), OUT_DT,
                           kind="ExternalOutput")

    with ExitStack() as ctx:
        tc = ctx.enter_context(tile.TileContext(nc))
        xp = ctx.enter_context(tc.tile_pool(name="xp", bufs=1))
        wp = ctx.enter_context(tc.tile_pool(name="wp", bufs=1))
        op = ctx.enter_context(tc.tile_pool(name="op", bufs=1))
        pp = ctx.enter_context(tc.tile_pool(name="pp", bufs=1, space="PSUM"))

        # PSUM: one full bank per frame-tile + one warm-up scratch bank
        ps = [pp.tile([128, 512], mybir.dt.float32, name=f"ps{t}",
                      tag=f"ps{t}") for t in range(NTILE)]
        pw = pp.tile([128, 512], mybir.dt.float32, name="pw", tag="pw")

        warm_sb = xp.tile([128, max(WARM_N, 128)], MM_DT, name="warm",
                          tag="warm")
        nc.vector.memset(warm_sb[:].bitcast(mybir.dt.float32), 0.0)
        for _ in range(N_WARM):
            nc.tensor.matmul(pw[:, 0:WARM_N], warm_sb[:, 0:128],
                             warm_sb[:, 0:WARM_N],
                             start=True, stop=True, skip_group_check=True)

        # --- SBUF tiles + input DMA streams ---
        xts = [xp.tile([128, MB2], MM_DT, name=f"x{rc}", tag=f"x{rc}")
               for rc in range(4)]
        x8s = [xp.tile([128, 2, MB2P], F8_DT, name=f"x8_{u}", tag=f"x8_{u}")
               for u in range(2)]
        wcs = wp.tile([128, S], MM_DT, name="wc_sb", tag="wc_sb")
        w8s = wp.tile([128, 2, S8], F8_DT, name="w8_sb", tag="w8_sb")

        qs = {"sync": nc.sync, "scalar": nc.scalar, "gpsimd": nc.gpsimd}
        for q, tn, lo, hi in w_pieces:
            if hi <= lo:
                continue
            if tn == "wc":
                qs[q].dma_start(wcs[:, lo:hi], wc_d[:, lo:hi])
            else:
                qs[q].dma_start(w8s[:, :, lo:hi], w8_d[:, :, lo:hi])
        for q, tn, idx, lo, hi in x_pieces:
            if tn == "xt":
                qs[q].dma_start(xts[idx][:, lo:hi], xt_d[idx][:, lo:hi])
            else:
                qs[q].dma_start(x8s[idx][:, :, lo:hi], x8_d[idx][:, :, lo:hi])

        # --- matmul streams: one per frame-tile ---
        DR = mybir.MatmulPerfMode.DoubleRow

        def emit_stream(t):
            v0, m = V0[t], MT[t]
            emits = [("mid", c) for c in p["order_mid"]]
            for q in p["pair_order"]:
                if p8[q] > 0:
                    emits.append(("pre", q))
                if mcols[2 * q] > s8[q]:
                    emits.append(("suf", q))
            emits = [e for e in emits
                     if e[0] != "mid" or mid_range(e[1])[1] > mid_range(e[1])[0]]
            first = True
            for n, (kind, v) in enumerate(emits):
                last = n == len(emits) - 1
                if kind == "mid":
                    c = v
                    lo, hi = mid_range(c)
                    j, rc = divmod(c, 4)
                    nc.tensor.matmul(
                        ps[t][0:m, lo:hi],
                        xts[rc][:, v0 + j:v0 + j + m],
                        wcs[:, woff[c]:woff[c] + hi - lo],
                        start=first, stop=last, skip_group_check=True)
                else:
                    q = v
                    u, jp = q % 2, q // 2
                    if kind == "pre":
                        cols = (0, int(p8[q]))
                        o8 = int(w8off_pre[q])
                    else:
                        cols = (int(s8[q]), int(mcols[2 * q]))
                        o8 = int(w8off_suf[q])
                    n8 = cols[1] - cols[0]
                    nc.tensor.matmul(
                        ps[t][0:m, cols[0]:cols[1]],
                        x8s[u][:, :, v0 + jp:v0 + jp + m],
                        w8s[:, :, o8:o8 + n8],
                        perf_mode=DR,
                        start=first, stop=last, skip_group_check=True)
                first = False

        split_last = bool(int(_os.environ.get("K_SPLITCOPY", "0")))
        ots = [op.tile([128, NROWS], OUT_DT, name=f"o{t}", tag=f"o{t}")
               for t in range(NTILE)]
        for t in range(NTILE):
            emit_stream(t)
            m = MT[t]
            if split_last and t == NTILE - 1:
                # halve the critical-path copy: DVE and Act each move half
                # the columns in parallel, then one DMA ships both
                nc.vector.tensor_copy(ots[t][0:m, 0:84], ps[t][0:m, 0:84])
                nc.scalar.copy(ots[t][0:m, 84:NROWS], ps[t][0:m, 84:NROWS])
            else:
                nc.vector.tensor_copy(ots[t][0:m, :], ps[t][0:m, 0:NROWS])
            nc.sync.dma_start(out_d[t, 0:m, :], ots[t][0:m, :])
    nc.finalize()
    return nc


def get_nc():
    global _NC
    if _NC is None:
        _NC = _build_nc()
    return _NC


def _pack_x(xb):
    """(C, T) -> fp16 (4, 128, MB2) + fp8 (2, 128, 2, MB2).

    xt[rc, r, m]    = xcat[m*512 + rc*128 + r]
    x8[u, r, i, m]  = xcat[m*512 + u*256 + i*128 + r]
    xcat = [ch0 blocks 0..430 | ch1 blocks 0..430], zero-padded tails."""
    xpad = np.zeros((C, MBLK * 512), dtype=np.float32)
    xpad[:, :T] = xb
    xcat = xpad.reshape(MB2, 512)
    xt = np.ascontiguousarray(
        xcat.reshape(MB2, 4, 128).transpose(1, 2, 0)).astype(np.float16)
    x8 = np.zeros((2, 128, 2, MB2P), dtype=F8_NP)
    x8[:, :, :, :MB2] = np.ascontiguousarray(
        xcat.reshape(MB2, 2, 2, 128).transpose(1, 3, 2, 0)).astype(F8_NP)
    return xt, x8


def kernel(x):
    global LAST_RESULTS
    x = np.asarray(x, dtype=np.float32)
    assert x.shape == (B, C, T)
    p = _params()
    in_maps = []
    for b in range(B):
        xt, x8 = _pack_x(x[b])
        in_maps.append({"xt": xt, "x8": x8, "wc": p["wc"], "w8": p["w8"]})
    nc = get_nc()
    res = run_bass_kernel_spmd(nc, in_maps, core_ids=list(range(N_CORES)))
    LAST_RESULTS = res
    out = np.empty((B, C, N_BINS, F, 2), dtype=np.float32)
    for b in range(B):
        raw = np.asarray(res.results[b]["out"])  # (NTILE, 128, NROWS)
        out[b] = _unpack_out(raw)
    return out


def _unpack_out(raw):
    """(NTILE, 128, NROWS) -> (C, N_BINS, F, 2)."""
    raw = np.asarray(raw, dtype=np.float32)
    cat = raw.reshape(NTILE * 128, NROWS)[:V0[-1] + MT[-1]]  # (840, 168)
    o = np.empty((C, N_BINS, F, 2), dtype=np.float32)
    o[0] = cat[0:F].reshape(F, N_BINS, 2).transpose(1, 0, 2)
    o[1] = cat[MBLK:MBLK + F].reshape(F, N_BINS, 2).transpose(1, 0, 2)
    return o


# revision 33
# speedup vs baseline: 3.6705x; 1.0007x over previous
"""Trainium2 Bass kernel for a CQT (constant-Q transform) nn.Module.

Reference computation (per batch sample b, channel c):
    out[b, c, k, f, 0] = sum_t x[b, c, f*HOP + t] * w_re[k, t]
    out[b, c, k, f, 1] = sum_t x[b, c, f*HOP + t] * w_im[k, t]
where w_re/w_im are Hann-windowed complex exponentials with per-bin ragged
lengths (longest 11340 samples), HOP=512, 84 bins, 409 frames.

Strategy: data-parallel over the batch (1 sample per NeuronCore, 8 cores).
Per core the PE matmuls put FRAMES on the output partition axis (stationary
operand = a 128-column slice of the resident signal tile) and the 168
interleaved (re,im) bin rows on the moving free axis.  The contraction axis t
is split into 89 chunks of 128; chunk c only involves the 2*n_act[c] rows of
bins whose window extends past 128*c, so each chunk's matmul moves just that
many rows -- the ragged bin lengths prune the work and the stationary
(weight-load) side is pipelined by the PE.

Both channels' frames are concatenated on one virtual frame axis (ch0 blocks
0..430, ch1 blocks 431..861 of the same 512-sample block grid), which lets
7 frame-tiles of 128 cover all 2*409 frames; tile 3 straddles the channel
seam (its middle 22 partitions compute junk that is never written out) and
tile 6 has only 72 live frames.

Precision split: the Hann window edges (t/L < TH_LO or > TH_HI) carry ~8%
of each window's energy but ~1/3 of the matmul rows.  Those column ranges
run as fp8e4 DoubleRow matmuls -- each covers a PAIR of 128-chunks (K=256)
at 0.5 cycles/row, a 4x throughput vs fp16 -- while the energetic window
middles stay fp16.  The fp8 dual-row Ldweights ISA requires the stationary
operand's plane stride to be a multiple of 4, hence the padded x8 layout.
Measured end-to-end relative error ~1.0e-2 (tolerance 2e-2).

Per stream, fp16 mids are emitted in rc-major rounds (rc = chunk%4 picks
the stationary signal tile), then all fp8 pairs -- so signal tiles and the
weight arrays (laid out in emission order) stream in while the first
frame-tile computes.  The Tile scheduler reorders matmuls across streams
by readiness, so the DMA piece split/ordering and the warm-up count are
tuned empirically against the CoreSim schedule (dummy warm-up matmuls also
bridge the PE p-state ramp during the initial DMA latency; an idle PE gap
before the first real matmul would both re-slow the ramp and shift the
scheduler into a worse semaphore batching).  PSUM accumulates in fp32, one
bank per frame-tile plus a warm-up scratch bank.
"""

import math
import os as _os
from contextlib import ExitStack

import ml_dtypes
import numpy as np

import concourse.bass as bass
import concourse.mybir as mybir
import concourse.tile as tile
from concourse import bacc
from concourse.bass_utils import run_bass_kernel_spmd

# ---- problem constants (hardcoded CQT spec) ----
SR = 22050
N_BINS = 84
BPO = 12
FMIN = 32.7
HOP = 512
B, C, T = 8, 2, 220500
N_CORES = 8

LMAX = 11340            # longest window
F = 409                 # frames: 1 + (T - LMAX)//HOP
NCHUNK = 89             # ceil(LMAX/128) contraction chunks
NPAIR = 44              # fp8 DoubleRow chunk pairs (0,1)..(86,87)
NROWS = 2 * N_BINS      # interleaved (re, im) weight rows
MBLK = 431              # 512-sample blocks per channel (ceil(220500/512))
MB2 = 2 * MBLK          # concatenated block axis (ch0 | ch1)
MB2P = 864              # x8 inner width: fp8 dual-row Ldweights requires the
                        # plane stride to be a multiple of 4 (862 -> pad 864)
NTILE = 7               # frame tiles of 128 over the 840-virtual-frame axis
V0 = [0, 128, 256, 384, 512, 640, 768]
MT = [128, 128, 128, 128, 128, 128, 72]  # live partition count per tile

MM_DT = mybir.dt.float16
F8_DT = mybir.dt.float8e4
OUT_DT = mybir.dt.float16  # staging/output dtype (host casts back to f32)
F8_NP = ml_dtypes.float8_e4m3

TH_LO = float(_os.environ.get("K_THLO", "0.32"))  # fp8 window-start region
TH_HI = float(_os.environ.get("K_THHI", "0.70"))  # fp8 window-tail region
N_WARM = int(_os.environ.get("K_NWARM", "3"))    # warm-up matmuls
WARM_N = int(_os.environ.get("K_WARMN", "116"))  # their moving size

_PREP = None
_NC = None
LAST_RESULTS = None


def _params():
    """Host-side constants: chunk geometry, fp8 pair selection, and
    emission-order weight layouts."""
    global _PREP
    if _PREP is not None:
        return _PREP

    Q = 1.0 / (2.0 ** (1.0 / BPO) - 1.0)
    freqs = FMIN * 2.0 ** (np.arange(N_BINS, dtype=np.float64) / BPO)
    lengths = np.round(Q * SR / freqs).astype(np.int64)
    assert int(lengths.max()) == LMAX

    t = np.arange(LMAX, dtype=np.float64)
    L = lengths.astype(np.float64)[:, None]
    mask = (t[None, :] < L).astype(np.float64)
    win = 0.5 * (1.0 - np.cos(2.0 * math.pi * t[None, :] / L)) * mask
    phase = (2.0 * math.pi / SR) * freqs[:, None] * t[None, :]
    w_re = (win * np.cos(phase)).astype(np.float32)
    w_im = (-win * np.sin(phase)).astype(np.float32)

    # rows 2k / 2k+1 = re_k / im_k; zero-pad time to NCHUNK*128
    W = np.zeros((NROWS, NCHUNK * 128), dtype=np.float32)
    W[0::2, :LMAX] = w_re
    W[1::2, :LMAX] = w_im
    WT = np.ascontiguousarray(W.T)  # (NCHUNK*128, NROWS)

    n_act = np.array([(lengths > 128 * c).sum() for c in range(NCHUNK)])
    assert n_act[0] == N_BINS and n_act[-1] >= 1
    mcols = (2 * n_act).astype(np.int64)  # active rows per chunk

    # fp8 pair selection: pair q = chunks (2q, 2q+1), samples [256q, 256q+256)
    # prefix cols [0:p8): bins whose window-start region contains the pair
    # suffix cols [s8:mcols[2q]): bins whose window-tail region contains it
    p8 = np.zeros(NPAIR, dtype=np.int64)
    s8 = np.zeros(NPAIR, dtype=np.int64)
    for q in range(NPAIR):
        lo, hi = 256 * q, 256 * (q + 1)
        p8[q] = 2 * int((lengths >= hi / TH_LO).sum()) if TH_LO > 0 else 0
        n_not_suf = int((lengths * TH_HI > lo).sum())
        s8[q] = 2 * max(n_not_suf, p8[q] // 2)
        s8[q] = min(s8[q], mcols[2 * q])
        p8[q] = min(p8[q], s8[q])
    # pair 0 keeps no fp8 prefix: chunk 0's full-width fp16 matmul must be
    # the stream's first write so its start=True arms the whole PSUM row and
    # every later (narrower) write lands on already-written bytes -- the
    # functional sim asserts uniform pending-zero state per matmul.
    p8[0] = 0
    assert s8[0] == mcols[0]

    def mid_range(c):
        if c == NCHUNK - 1:
            return (0, int(mcols[c]))
        q = c // 2
        lo = int(p8[q])
        hi = int(s8[q]) if c % 2 == 0 else min(int(s8[q]), int(mcols[c]))
        return (lo, max(lo, hi))

    # emission: rc-major rounds of fp16 mids, then all fp8 pairs ascending
    # (their inputs stream in while the mids run)
    order_mid = [c for r in range(4) for c in range(r, NCHUNK, 4)]
    pair_order = list(range(NPAIR))

    # fp16 weight layout: mid blocks in emission order, column-compacted
    woff = {}
    off = 0
    for c in order_mid:
        lo, hi = mid_range(c)
        woff[c] = off
        off += hi - lo
    S = int(off)
    wc = np.zeros((128, S), dtype=np.float16)
    for c in order_mid:
        lo, hi = mid_range(c)
        if hi > lo:
            wc[:, woff[c]:woff[c] + hi - lo] = \
                WT[128 * c:128 * (c + 1), lo:hi].astype(np.float16)

    # fp8 weight layout: pairs in emission order, prefix block then suffix
    # block per pair; plane i = chunk 2q+i
    w8off_pre = np.zeros(NPAIR, dtype=np.int64)
    w8off_suf = np.zeros(NPAIR, dtype=np.int64)
    off = 0
    for q in pair_order:
        w8off_pre[q] = off
        off += int(p8[q])
        w8off_suf[q] = off
        off += int(mcols[2 * q] - s8[q])
    S8 = int(off + (-off) % 4)  # fp8 dual-row plane stride must be 4-aligned
    w8 = np.zeros((128, 2, S8), dtype=np.float32)
    for q in range(NPAIR):
        m0, s, pq = int(mcols[2 * q]), int(s8[q]), int(p8[q])
        for i in range(2):
            blk = WT[128 * (2 * q + i):128 * (2 * q + i + 1), :]
            if pq:
                w8[:, i, w8off_pre[q]:w8off_pre[q] + pq] = blk[:, :pq]
            if m0 > s:
                w8[:, i, w8off_suf[q]:w8off_suf[q] + m0 - s] = blk[:, s:m0]
    w8 = w8.astype(F8_NP)

    _PREP = dict(mcols=mcols, p8=p8, s8=s8, mid_range=mid_range,
                 order_mid=order_mid, pair_order=pair_order,
                 woff=woff, S=S, wc=wc,
                 w8off_pre=w8off_pre, w8off_suf=w8off_suf, S8=S8, w8=w8)
    return _PREP


def _dma_plan(p):
    """(queue, tensor, slice) pieces, in per-queue emission order."""
    S, S8 = p["S"], p["S8"]
    order_mid, mid_range = p["order_mid"], p["mid_range"]
    # fp16 weight column position at each rc-round boundary
    r_end = []
    off = 0
    for r in range(4):
        for c in range(r, NCHUNK, 4):
            lo, hi = mid_range(c)
            off += hi - lo
        r_end.append(off)
    w0_mid = r_end[0] // 2
    w8_mid = S8 // 2
    plan = int(_os.environ.get("K_PLAN", "4"))

    if plan == 0:
        w_pieces = [
            ("sync", "wc", 0, w0_mid),
            ("sync", "wc", w0_mid, r_end[0]),
            ("sync", "wc", r_end[0], r_end[1]),
            ("sync", "wc", r_end[1], r_end[2]),
            ("sync", "wc", r_end[2], r_end[3]),
            ("sync", "w8", 0, w8_mid),
            ("sync", "w8", w8_mid, S8),
        ]
        x_pieces = [
            ("scalar", "xt", 0, 0, 288),
            ("scalar", "xt", 1, 0, 288),
            ("gpsimd", "xt", 2, 0, 288),
            ("gpsimd", "xt", 3, 0, 288),
            ("scalar", "x8", 0, 0, 288),
            ("gpsimd", "x8", 1, 0, 288),
            ("scalar", "xt", 0, 288, MB2),
            ("scalar", "xt", 1, 288, MB2),
            ("gpsimd", "xt", 2, 288, MB2),
            ("gpsimd", "xt", 3, 288, MB2),
            ("scalar", "x8", 0, 288, MB2),
            ("gpsimd", "x8", 1, 288, MB2),
        ]
    elif plan == 1:  # x remainders early, fp8 data late
        w_pieces = [
            ("sync", "wc", 0, w0_mid),
            ("sync", "wc", w0_mid, r_end[0]),
            ("sync", "wc", r_end[0], r_end[1]),
            ("sync", "wc", r_end[1], r_end[2]),
            ("sync", "wc", r_end[2], r_end[3]),
            ("sync", "w8", 0, S8),
        ]
        x_pieces = [
            ("scalar", "xt", 0, 0, 288),
            ("scalar", "xt", 1, 0, 288),
            ("gpsimd", "xt", 2, 0, 288),
            ("gpsimd", "xt", 3, 0, 288),
            ("scalar", "xt", 0, 288, 576),
            ("scalar", "xt", 1, 288, 576),
            ("gpsimd", "xt", 2, 288, 576),
            ("gpsimd", "xt", 3, 288, 576),
            ("scalar", "xt", 0, 576, MB2),
            ("scalar", "xt", 1, 576, MB2),
            ("gpsimd", "xt", 2, 576, MB2),
            ("gpsimd", "xt", 3, 576, MB2),
            ("scalar", "x8", 0, 0, MB2),
            ("gpsimd", "x8", 1, 0, MB2),
        ]
    elif plan == 2:  # interleave x remainders right after windows per queue
        w_pieces = [
            ("sync", "wc", 0, r_end[0]),
            ("sync", "wc", r_end[0], r_end[1]),
            ("sync", "wc", r_end[1], r_end[2]),
            ("sync", "wc", r_end[2], r_end[3]),
            ("sync", "w8", 0, S8),
        ]
        x_pieces = [
            ("scalar", "xt", 0, 0, 288),
            ("scalar", "xt", 1, 0, 288),
            ("gpsimd", "xt", 2, 0, 288),
            ("gpsimd", "xt", 3, 0, 288),
            ("scalar", "xt", 1, 288, 576),
            ("scalar", "xt", 0, 288, 576),
            ("gpsimd", "xt", 3, 288, 576),
            ("gpsimd", "xt", 2, 288, 576),
            ("scalar", "xt", 0, 576, MB2),
            ("scalar", "xt", 1, 576, MB2),
            ("gpsimd", "xt", 2, 576, MB2),
            ("gpsimd", "xt", 3, 576, MB2),
            ("scalar", "x8", 0, 0, MB2),
            ("gpsimd", "x8", 1, 0, MB2),
        ]
    elif plan == 3:  # everything big, fewer pieces
        w_pieces = [
            ("sync", "wc", 0, r_end[1]),
            ("sync", "wc", r_end[1], r_end[3]),
            ("sync", "w8", 0, S8),
        ]
        x_pieces = [
            ("scalar", "xt", 0, 0, MB2),
            ("scalar", "xt", 1, 0, MB2),
            ("gpsimd", "xt", 2, 0, MB2),
            ("gpsimd", "xt", 3, 0, MB2),
            ("scalar", "x8", 0, 0, MB2),
            ("gpsimd", "x8", 1, 0, MB2),
        ]
    elif plan == 4:  # plan2 with w8 earlier on sync
        w_pieces = [
            ("sync", "wc", 0, r_end[0]),
            ("sync", "wc", r_end[0], r_end[1]),
            ("sync", "w8", 0, w8_mid),
            ("sync", "wc", r_end[1], r_end[2]),
            ("sync", "wc", r_end[2], r_end[3]),
            ("sync", "w8", w8_mid, S8),
        ]
        x_pieces = [
            ("scalar", "xt", 0, 0, 288),
            ("scalar", "xt", 1, 0, 288),
            ("gpsimd", "xt", 2, 0, 288),
            ("gpsimd", "xt", 3, 0, 288),
            ("scalar", "xt", 1, 288, 576),
            ("scalar", "xt", 0, 288, 576),
            ("gpsimd", "xt", 3, 288, 576),
            ("gpsimd", "xt", 2, 288, 576),
            ("scalar", "xt", 0, 576, MB2),
            ("scalar", "xt", 1, 576, MB2),
            ("gpsimd", "xt", 2, 576, MB2),
            ("gpsimd", "xt", 3, 576, MB2),
            ("scalar", "x8", 0, 0, MB2),
            ("gpsimd", "x8", 1, 0, MB2),
        ]
    elif plan == 5:  # plan2 with x8 split windows early
        w_pieces = [
            ("sync", "wc", 0, r_end[0]),
            ("sync", "wc", r_end[0], r_end[1]),
            ("sync", "wc", r_end[1], r_end[2]),
            ("sync", "wc", r_end[2], r_end[3]),
            ("sync", "w8", 0, S8),
        ]
        x_pieces = [
            ("scalar", "xt", 0, 0, 288),
            ("scalar", "xt", 1, 0, 288),
            ("gpsimd", "xt", 2, 0, 288),
            ("gpsimd", "xt", 3, 0, 288),
            ("scalar", "x8", 0, 0, 288),
            ("gpsimd", "x8", 1, 0, 288),
            ("scalar", "xt", 1, 288, 576),
            ("scalar", "xt", 0, 288, 576),
            ("gpsimd", "xt", 3, 288, 576),
            ("gpsimd", "xt", 2, 288, 576),
            ("scalar", "xt", 0, 576, MB2),
            ("scalar", "xt", 1, 576, MB2),
            ("gpsimd", "xt", 2, 576, MB2),
            ("gpsimd", "xt", 3, 576, MB2),
            ("scalar", "x8", 0, 288, MB2),
            ("gpsimd", "x8", 1, 288, MB2),
        ]
    elif plan == 6:  # plan2 with 256-col windows
        w_pieces = [
            ("sync", "wc", 0, r_end[0]),
            ("sync", "wc", r_end[0], r_end[1]),
            ("sync", "wc", r_end[1], r_end[2]),
            ("sync", "wc", r_end[2], r_end[3]),
            ("sync", "w8", 0, S8),
        ]
        x_pieces = [
            ("scalar", "xt", 0, 0, 256),
            ("scalar", "xt", 1, 0, 256),
            ("gpsimd", "xt", 2, 0, 256),
            ("gpsimd", "xt", 3, 0, 256),
            ("scalar", "xt", 1, 256, 576),
            ("scalar", "xt", 0, 256, 576),
            ("gpsimd", "xt", 3, 256, 576),
            ("gpsimd", "xt", 2, 256, 576),
            ("scalar", "xt", 0, 576, MB2),
            ("scalar", "xt", 1, 576, MB2),
            ("gpsimd", "xt", 2, 576, MB2),
            ("gpsimd", "xt", 3, 576, MB2),
            ("scalar", "x8", 0, 0, MB2),
            ("gpsimd", "x8", 1, 0, MB2),
        ]
    else:  # plan 7: plan2 with wc r0 split
        w_pieces = [
            ("sync", "wc", 0, w0_mid),
            ("sync", "wc", w0_mid, r_end[0]),
            ("sync", "wc", r_end[0], r_end[1]),
            ("sync", "wc", r_end[1], r_end[2]),
            ("sync", "wc", r_end[2], r_end[3]),
            ("sync", "w8", 0, S8),
        ]
        x_pieces = [
            ("scalar", "xt", 0, 0, 288),
            ("scalar", "xt", 1, 0, 288),
            ("gpsimd", "xt", 2, 0, 288),
            ("gpsimd", "xt", 3, 0, 288),
            ("scalar", "xt", 1, 288, 576),
            ("scalar", "xt", 0, 288, 576),
            ("gpsimd", "xt", 3, 288, 576),
            ("gpsimd", "xt", 2, 288, 576),
            ("scalar", "xt", 0, 576, MB2),
            ("scalar", "xt", 1, 576, MB2),
            ("gpsimd", "xt", 2, 576, MB2),
            ("gpsimd", "xt", 3, 576, MB2),
            ("scalar", "x8", 0, 0, MB2),
            ("gpsimd", "x8", 1, 0, MB2),
        ]
    return w_pieces, x_pieces


def _build_nc():
    p = _params()
    mcols, mid_range = p["mcols"], p["mid_range"]
    woff, S = p["woff"], p["S"]
    p8, s8 = p["p8"], p["s8"]
    w8off_pre, w8off_suf, S8 = p["w8off_pre"], p["w8off_suf"], p["S8"]
    w_pieces, x_pieces = _dma_plan(p)

    nc = bacc.Bacc(None, target_bir_lowering=False)
    xt_d = nc.dram_tensor("xt", (4, 128, MB2), MM_DT, kind="ExternalInput")
    x8_d = nc.dram_tensor("x8", (2, 128, 2, MB2P), F8_DT, kind="ExternalInput")
    wc_d = nc.dram_tensor("wc", (128, S), MM_DT, kind="ExternalInput")
    w8_d = nc.dram_tensor("w8", (128, 2, S8), F8_DT, kind="ExternalInput")
    out_d = nc.dram_tensor("out", (NTILE, 128, NROWS), OUT_DT,
                           kind="ExternalOutput")

    with ExitStack() as ctx:
        tc = ctx.enter_context(tile.TileContext(nc))
        xp = ctx.enter_context(tc.tile_pool(name="xp", bufs=1))
        wp = ctx.enter_context(tc.tile_pool(name="wp", bufs=1))
        op = ctx.enter_context(tc.tile_pool(name="op", bufs=1))
        pp = ctx.enter_context(tc.tile_pool(name="pp", bufs=1, space="PSUM"))

        # PSUM: one full bank per frame-tile + one warm-up scratch bank
        ps = [pp.tile([128, 512], mybir.dt.float32, name=f"ps{t}",
                      tag=f"ps{t}") for t in range(NTILE)]
        pw = pp.tile([128, 512], mybir.dt.float32, name="pw", tag="pw")

        warm_sb = xp.tile([128, max(WARM_N, 128)], MM_DT, name="warm",
                          tag="warm")
        nc.vector.memset(warm_sb[:].bitcast(mybir.dt.float32), 0.0)
        for _ in range(N_WARM):
            nc.tensor.matmul(pw[:, 0:WARM_N], warm_sb[:, 0:128],
                             warm_sb[:, 0:WARM_N],
                             start=True, stop=True, skip_group_check=True)

        # --- SBUF tiles + input DMA streams ---
        xts = [xp.tile([128, MB2], MM_DT, name=f"x{rc}", tag=f"x{rc}")
               for rc in range(4)]
        x8s = [xp.tile([128, 2, MB2P], F8_DT, name=f"x8_{u}", tag=f"x8_{u}")
               for u in range(2)]
        wcs = wp.tile([128, S], MM_DT, name="wc_sb", tag="wc_sb")
        w8s = wp.tile([128, 2, S8], F8_DT, name="w8_sb", tag="w8_sb")

        qs = {"sync": nc.sync, "scalar": nc.scalar, "gpsimd": nc.gpsimd}
        for q, tn, lo, hi in w_pieces:
            if hi <= lo:
                continue
            if tn == "wc":
                qs[q].dma_start(wcs[:, lo:hi], wc_d[:, lo:hi])
            else:
                qs[q].dma_start(w8s[:, :, lo:hi], w8_d[:, :, lo:hi])
        for q, tn, idx, lo, hi in x_pieces:
            if tn == "xt":
                qs[q].dma_start(xts[idx][:, lo:hi], xt_d[idx][:, lo:hi])
            else:
                qs[q].dma_start(x8s[idx][:, :, lo:hi], x8_d[idx][:, :, lo:hi])

        # --- matmul streams: one per frame-tile ---
        DR = mybir.MatmulPerfMode.DoubleRow

        def emit_stream(t):
            v0, m = V0[t], MT[t]
            emits = [("mid", c) for c in p["order_mid"]]
            for q in p["pair_order"]:
                if p8[q] > 0:
                    emits.append(("pre", q))
                if mcols[2 * q] > s8[q]:
                    emits.append(("suf", q))
            emits = [e for e in emits
                     if e[0] != "mid" or mid_range(e[1])[1] > mid_range(e[1])[0]]
            first = True
            for n, (kind, v) in enumerate(emits):
                last = n == len(emits) - 1
                if kind == "mid":
                    c = v
                    lo, hi = mid_range(c)
                    j, rc = divmod(c, 4)
                    nc.tensor.matmul(
                        ps[t][0:m, lo:hi],
                        xts[rc][:, v0 + j:v0 + j + m],
                        wcs[:, woff[c]:woff[c] + hi - lo],
                        start=first, stop=last, skip_group_check=True)
                else:
                    q = v
                    u, jp = q % 2, q // 2
                    if kind == "pre":
                        cols = (0, int(p8[q]))
                        o8 = int(w8off_pre[q])
                    else:
                        cols = (int(s8[q]), int(mcols[2 * q]))
                        o8 = int(w8off_suf[q])
                    n8 = cols[1] - cols[0]
                    nc.tensor.matmul(
                        ps[t][0:m, cols[0]:cols[1]],
                        x8s[u][:, :, v0 + jp:v0 + jp + m],
                        w8s[:, :, o8:o8 + n8],
                        perf_mode=DR,
                        start=first, stop=last, skip_group_check=True)
                first = False

        split_last = bool(int(_os.environ.get("K_SPLITCOPY", "0")))
        ots = [op.tile([128, NROWS], OUT_DT, name=f"o{t}", tag=f"o{t}")
               for t in range(NTILE)]
        for t in range(NTILE):
            emit_stream(t)
            m = MT[t]
            if split_last and t == NTILE - 1:
                # halve the critical-path copy: DVE and Act each move half
                # the columns in parallel, then one DMA ships both
                nc.vector.tensor_copy(ots[t][0:m, 0:84], ps[t][0:m, 0:84])
                nc.scalar.copy(ots[t][0:m, 84:NROWS], ps[t][0:m, 84:NROWS])
            else:
                nc.vector.tensor_copy(ots[t][0:m, :], ps[t][0:m, 0:NROWS])
            nc.sync.dma_start(out_d[t, 0:m, :], ots[t][0:m, :])
    nc.finalize()
    return nc


def get_nc():
    global _NC
    if _NC is None:
        _NC = _build_nc()
    return _NC


def _pack_x(xb):
    """(C, T) -> fp16 (4, 128, MB2) + fp8 (2, 128, 2, MB2).

    xt[rc, r, m]    = xcat[m*512 + rc*128 + r]
    x8[u, r, i, m]  = xcat[m*512 + u*256 + i*128 + r]
    xcat = [ch0 blocks 0..430 | ch1 blocks 0..430], zero-padded tails."""
    xpad = np.zeros((C, MBLK * 512), dtype=np.float32)
    xpad[:, :T] = xb
    xcat = xpad.reshape(MB2, 512)
    xt = np.ascontiguousarray(
        xcat.reshape(MB2, 4, 128).transpose(1, 2, 0)).astype(np.float16)
    x8 = np.zeros((2, 128, 2, MB2P), dtype=F8_NP)
    x8[:, :, :, :MB2] = np.ascontiguousarray(
        xcat.reshape(MB2, 2, 2, 128).transpose(1, 3, 2, 0)).astype(F8_NP)
    return xt, x8


def kernel(x):
    global LAST_RESULTS
    x = np.asarray(x, dtype=np.float32)
    assert x.shape == (B, C, T)
    p = _params()
    in_maps = []
    for b in range(B):
        xt, x8 = _pack_x(x[b])
        in_maps.append({"xt": xt, "x8": x8, "wc": p["wc"], "w8": p["w8"]})
    nc = get_nc()
    res = run_bass_kernel_spmd(nc, in_maps, core_ids=list(range(N_CORES)))
    LAST_RESULTS = res
    out = np.empty((B, C, N_BINS, F, 2), dtype=np.float32)
    for b in range(B):
        raw = np.asarray(res.results[b]["out"])  # (NTILE, 128, NROWS)
        out[b] = _unpack_out(raw)
    return out


def _unpack_out(raw):
    """(NTILE, 128, NROWS) -> (C, N_BINS, F, 2)."""
    raw = np.asarray(raw, dtype=np.float32)
    cat = raw.reshape(NTILE * 128, NROWS)[:V0[-1] + MT[-1]]  # (840, 168)
    o = np.empty((C, N_BINS, F, 2), dtype=np.float32)
    o[0] = cat[0:F].reshape(F, N_BINS, 2).transpose(1, 0, 2)
    o[1] = cat[MBLK:MBLK + F].reshape(F, N_BINS, 2).transpose(1, 0, 2)
    return o
